# revision 1
# baseline (speedup 1.0000x reference)
"""Causal self-attention (B=2, T=2048, D=1024, H=16, DH=64) on 8 trn2 cores.

Sharding: DP on batch (2) x TP on heads (4 heads/core). Each core computes
qkv for its heads from x[b]^T, RoPE, causal SDPA, and a partial row-parallel
output projection y^T [D, T]. Host sums TP partials, transposes, adds bias.

Inputs (x, Wqkv, Wv, Wproj) and the whole attention stage run in bf16, which
keeps the PE at 1 cycle/row even for narrow diagonal tiles, halves DMA
traffic, and unlocks DVE 16-bit modes; PSUM accumulation stays fp32 and the
final rel-err is ~5e-3 against the fp32 reference. Per (q-chunk, head-pair)
the even/odd-head score tiles land in one 2-bank PSUM tile so a single
activation instruction computes exp for both heads (PSUM accumulation groups
must never share a bank). The softmax denominator rides the AV matmul as an
extra ones-column; 1/Z is partition-broadcast via DVE stream_shuffles, the
unnormalized y is staged to SBUF so the AV PSUM slot frees early, and the
final scale runs on the (otherwise idle) gpsimd engine.

Chunk 0's qkv runs o-major so matmuls chase the startup DMA stream (paired
wqk/x/wv slices of the contraction dim; HWDGE descriptor generation at
~625ns/DMA is the startup pacer). Later chunks prefetch x and interleave
next-chunk qkv between a head pair's AV stream and its normalize so the
statically-scheduled PE stream never waits on softmax latency. The last
chunk's S/exp stream is hoisted into the previous chunk's phase (the tail is
activation-bound) and its projection is split into pt2 phases across all 8
PSUM banks so output copies/DMAs drain while hp1's normalize completes.
"""
import sys

if "/opt/trn_rl_repo" not in sys.path:
    sys.path.insert(0, "/opt/trn_rl_repo")

import numpy as np
import ml_dtypes

B, T, D = 2, 2048, 1024
H, DH = 16, 64
ROPE_BASE = 10000.0
NCORES = 8
TP = 4                # TP group size (cores per batch)
HL = H // TP          # heads per core = 4
CHUNK = 512           # t/q chunk
NCH = T // CHUNK      # 4
KT = 128              # k tile
NKT = T // KT         # 16
DIN = HL * DH         # 256 local head dims
SCALE = 1.0 / float(np.sqrt(DH))
NWARM = 36            # PE warmup matmuls (pstate ramp)

_compiled = None
_last_results = None


def _round_fp32r(x: np.ndarray) -> np.ndarray:
    u = np.ascontiguousarray(x, dtype=np.float32).view(np.uint32)
    u = (u + np.uint32(0x7FF) + ((u >> np.uint32(12)) & np.uint32(1))) & np.uint32(0xFFFFF000)
    return u.view(np.float32)


def _build(debug=False):
    import concourse.bass as bass
    import concourse.mybir as mybir
    import concourse.tile as tile
    from concourse import bacc

    F32 = mybir.dt.float32
    F32R = mybir.dt.float32r
    BF16 = mybir.dt.bfloat16
    ADD = mybir.AluOpType.add
    MULT = mybir.AluOpType.mult
    EXP = mybir.ActivationFunctionType.Exp

    nc = bacc.Bacc("TRN2", target_bir_lowering=False, num_devices=NCORES)

    xT = nc.dram_tensor("xT", [D, T], BF16, kind="ExternalInput")
    wqk = nc.dram_tensor("wqk", [D, 2 * DIN], BF16, kind="ExternalInput")
    wv = nc.dram_tensor("wv", [D, DIN], BF16, kind="ExternalInput")
    wproj = nc.dram_tensor("wproj", [DIN, D], BF16, kind="ExternalInput")
    bqk = nc.dram_tensor("bqk", [128, 4], F32, kind="ExternalInput")
    bv = nc.dram_tensor("bv", [128, DIN], F32, kind="ExternalInput")
    cos2 = nc.dram_tensor("cos2", [128, T], BF16, kind="ExternalInput")
    sin2 = nc.dram_tensor("sin2", [128, T], F32, kind="ExternalInput")
    perm = nc.dram_tensor("perm", [128, 128], BF16, kind="ExternalInput")
    vones = nc.dram_tensor("vones", [128, NKT, 2, 1], BF16, kind="ExternalInput")
    trimask = nc.dram_tensor("trimask", [128, 2, 128], BF16, kind="ExternalInput")
    yT = nc.dram_tensor("yT", [D, T], BF16, kind="ExternalOutput")
    if debug:
        dbg_qk = nc.dram_tensor("dbg_qk", [128, 4, T], BF16, kind="ExternalOutput")
        dbg_v = nc.dram_tensor("dbg_v", [128, NKT, 2, 192], BF16, kind="ExternalOutput")
        dbg_y = nc.dram_tensor("dbg_y", [128, 2, T], F32, kind="ExternalOutput")
        dbg_rr = nc.dram_tensor("dbg_rr", [96, CHUNK], F32, kind="ExternalOutput")
        dbg_bc = nc.dram_tensor("dbg_bc", [128, CHUNK], F32, kind="ExternalOutput")
        dbg_yc = nc.dram_tensor("dbg_yc", [128, CHUNK], F32, kind="ExternalOutput")

    xT3 = xT[:].rearrange("(o p) t -> p o t", p=128)
    wqk3 = wqk[:].rearrange("(o p) f -> p o f", p=128)
    wv3 = wv[:].rearrange("(o p) f -> p o f", p=128)

    with tile.TileContext(nc) as tc:
        with tc.tile_pool(name="const", bufs=1) as constp, \
             tc.tile_pool(name="big", bufs=1) as bigp, \
             tc.tile_pool(name="xin", bufs=3) as xinp, \
             tc.tile_pool(name="ptile", bufs=18) as ptp, \
             tc.tile_pool(name="tmp", bufs=3) as tmpp, \
             tc.tile_pool(name="rsm", bufs=4) as rsmp, \
             tc.tile_pool(name="outs", bufs=6) as outsp, \
             tc.tile_pool(name="psmm", bufs=2, space="PSUM") as psmm, \
             tc.tile_pool(name="pss", bufs=2, space="PSUM") as pss, \
             tc.tile_pool(name="psav", bufs=1, space="PSUM") as psav:

            # ---- persistent SBUF tensors ----
            wqk_sb = constp.tile([128, 8, 2 * DIN], BF16)     # [p, din_o, f]
            wv_sb = constp.tile([128, 8, DIN], BF16)
            wproj_sb = constp.tile([128, 2, D], BF16)         # [p, din_tile, dout]
            bqk_sb = constp.tile([128, 4], F32)
            bv_sb = constp.tile([128, DIN], F32)
            cos_sb = constp.tile([128, T], BF16)
            sin_sb = constp.tile([128, T], F32)
            perm_sb = constp.tile([128, 128], BF16)
            tri_sb = constp.tile([128, 2, 128], BF16)

            qk_sb = bigp.tile([128, 4, T], BF16)              # fb: q01,q23,k01,k23
            v_sb = bigp.tile([128, NKT, 2, 192], BF16)        # [t_p, kt, hp, cols]
            y_sb = bigp.tile([128, 2, T], BF16)               # y^T (din on partitions)

            # PE pstate warmup: dep-free matmul chain while startup DMAs land
            nc.sync.dma_start(perm_sb[:], perm[:])
            pwm = psmm.tile([128, 128], F32, tag="mm", name="pwm")
            for _ in range(NWARM):
                nc.tensor.matmul(pwm[:], perm_sb[:], perm_sb[:], start=True, stop=True)

            # startup DMAs in consumption order: o-pair triples of wqk/x0/wv
            # feed chunk 0's o-major qkv (HWDGE generation at 625ns/DMA is the
            # pacer, so two o-slices per DMA), then rope tables
            x_tiles = {}
            x_tiles[0] = xinp.tile([128, 8, CHUNK], BF16, tag="xchunk", name="x_c0")
            for o2 in range(4):
                o = bass.ds(2 * o2, 2)
                nc.sync.dma_start(wqk_sb[:, o], wqk3[:, o])
                nc.sync.dma_start(x_tiles[0][:, o], xT3[:, o, 0:CHUNK])
                nc.sync.dma_start(wv_sb[:, o], wv3[:, o])

            def emit_tail_dmas():
                nc.sync.dma_start(cos_sb[:, CHUNK:], cos2[:, CHUNK:])
                nc.sync.dma_start(sin_sb[:, CHUNK:], sin2[:, CHUNK:])
                nc.sync.dma_start(wproj_sb[:],
                                  wproj[:].rearrange("(o p) f -> p o f", p=128))

            def emit_rope(c, fb, qkpre):
                # roped = qkpre*cos + perm(qkpre)*sin, written to qk_sb
                cc = bass.ds(c * CHUNK, CHUNK)
                pp = psmm.tile([128, CHUNK], F32, tag="mm", name="pp")
                nc.tensor.matmul(pp[:], perm_sb[:], qkpre[:], start=True, stop=True)
                nc.vector.tensor_tensor(qk_sb[:, fb, cc], qkpre[:], cos_sb[:, cc], MULT)
                swapped = tmpp.tile([128, CHUNK], BF16, tag="rope")
                nc.vector.tensor_tensor(swapped[:], pp[:], sin_sb[:, cc], MULT)
                nc.vector.tensor_tensor(qk_sb[:, fb, cc], qk_sb[:, fb, cc], swapped[:], ADD)

            def emit_vwrite(kt, hp, pv):
                # pv cols [hp*128, hp*128+128) -> v_sb [0:64] and [128:192]
                vdst = v_sb[:, kt, hp].rearrange("p (b x) -> p b x", x=64)[:, 0::2]
                vsrc = pv[:, hp * 128:(hp + 1) * 128].rearrange("p (b x) -> p b x", x=64)
                bsrc = bv_sb[:, hp * 128:(hp + 1) * 128].rearrange("p (b x) -> p b x", x=64)
                nc.vector.tensor_tensor(vdst, vsrc, bsrc, ADD)

            def emit_qkv0():
                # chunk 0: o-major so matmuls chase the startup DMA stream
                x_sb = x_tiles.pop(0)
                # borrow the attention-stage pss slots (idle during startup):
                # two 2-bank slots hold the four q/k blocks, one per bank
                # (PSUM accumulation groups must not share a bank)
                psq = [pss.tile([128, 2, CHUNK], F32, tag="s", name=f"psq{i}")
                       for i in range(2)]
                pqs = [psq[fb // 2][:, fb % 2, :] for fb in range(4)]
                for o in range(8):
                    for fb in range(4):
                        nc.tensor.matmul(
                            pqs[fb], wqk_sb[:, o, fb * 128:(fb + 1) * 128],
                            x_sb[:, o], start=(o == 0), stop=(o == 7),
                            skip_group_check=True)
                def finish(fbs, tbs):
                    for fb in fbs:
                        qkpre = tmpp.tile([128, CHUNK], BF16, tag="qkpre")
                        nc.vector.tensor_scalar_add(qkpre[:], pqs[fb],
                                                    bqk_sb[:, fb:fb + 1])
                        emit_rope(0, fb, qkpre)
                    for tb in tbs:
                        pvfull = psmm.tile([128, CHUNK], F32, tag="mm",
                                           name="pvfull")
                        pv = pvfull[:, :DIN]
                        for o in range(8):
                            nc.tensor.matmul(
                                pv[:], x_sb[:, o, tb * 128:(tb + 1) * 128],
                                wv_sb[:, o], start=(o == 0), stop=(o == 7))
                        for hp in range(2):
                            emit_vwrite(tb, hp, pv[:])
                return finish

            def emit_qkv_qk(c, fbs=range(4)):
                # q^T,k^T for chunk c: [f, t], bias-add + rope per fb block
                cc = bass.ds(c * CHUNK, CHUNK)
                x_sb = x_tiles[c]
                for fb in fbs:
                    pq = psmm.tile([128, CHUNK], F32, tag="mm", name="pq")
                    for o in range(8):
                        nc.tensor.matmul(
                            pq[:], wqk_sb[:, o, fb * 128:(fb + 1) * 128], x_sb[:, o],
                            start=(o == 0), stop=(o == 7))
                    qkpre = tmpp.tile([128, CHUNK], BF16, tag="qkpre")
                    nc.vector.tensor_scalar_add(qkpre[:], pq[:], bqk_sb[:, fb:fb + 1])
                    emit_rope(c, fb, qkpre)

            def emit_qkv_v(c):
                x_sb = x_tiles.pop(c)
                for tb in range(4):
                    kt = c * 4 + tb
                    pvfull = psmm.tile([128, CHUNK], F32, tag="mm", name="pvfull")
                    pv = pvfull[:, :DIN]
                    for o in range(8):
                        nc.tensor.matmul(
                            pv[:], x_sb[:, o, tb * 128:(tb + 1) * 128], wv_sb[:, o],
                            start=(o == 0), stop=(o == 7))
                    for hp in range(2):
                        emit_vwrite(kt, hp, pv[:])

            def emit_xload(c, slices=1):
                if c < NCH:
                    x_tiles[c] = xinp.tile([128, 8, CHUNK], BF16, tag="xchunk",
                                           name=f"x_c{c}")
                    cc = bass.ds(c * CHUNK, CHUNK)
                    w = 8 // slices
                    for i in range(slices):
                        nc.sync.dma_start(x_tiles[c][:, i * w:(i + 1) * w],
                                          xT3[:, i * w:(i + 1) * w, cc])

            def emit_attn_S(c, hp):
                # S + exp + mask for q-chunk c, head pair hp; even head uses
                # PE rows 0-63 / psum par 0, odd head rows 64-127 / par 1.
                nkt_c = 4 * c + 4
                p_tiles = []
                for kt in range(nkt_c):
                    i = kt - 4 * c  # >=0 on diagonal tiles
                    col0 = 128 * i if i >= 0 else 0
                    ps = pss.tile([128, 2, CHUNK], F32, tag="s", name="ps")
                    for par in range(2):
                        base = 64 * par
                        nc.tensor.matmul(
                            ps[:, par, col0:],
                            qk_sb[base:base + 64, 2 + hp, kt * 128:(kt + 1) * 128],
                            qk_sb[base:base + 64, hp, bass.ds(c * CHUNK + col0,
                                                              CHUNK - col0)],
                            start=True, stop=True, skip_group_check=True)
                    pt = ptp.tile([128, 2, CHUNK], BF16, tag="p", name="pt")
                    nc.scalar.activation(pt[:, :, col0:], ps[:, :, col0:], EXP,
                                         bias=0.0, scale=SCALE)
                    if i >= 0:
                        # zero k>q entries of the diagonal block (0/1 mask);
                        # gpsimd (SBUF-only) keeps this off the busy DVE
                        nc.gpsimd.tensor_tensor(
                            pt[:, :, col0:col0 + 128], pt[:, :, col0:col0 + 128],
                            tri_sb[:], MULT)
                    p_tiles.append(pt)
                return p_tiles

            def emit_attn_AV(c, hp, p_tiles):
                nkt_c = 4 * c + 4
                pav = psav.tile([128, 2, CHUNK], F32, tag="av", name="pav")
                par_major = False
                rr2 = rsmp.tile([96, CHUNK], BF16, tag="r", name="rr2")
                bc_sb = rsmp.tile([128, CHUNK], BF16, tag="bc", name="bc_sb")
                av_order = ([(kt, par) for par in range(2) for kt in range(nkt_c)]
                            if par_major else
                            [(kt, par) for kt in range(nkt_c) for par in range(2)])
                for kt, par in av_order:
                    i = kt - 4 * c
                    col0 = 128 * i if i >= 0 else 0
                    pt = p_tiles[kt]
                    if par == 0:
                        nc.tensor.matmul(
                            pav[:65, 0, col0:], v_sb[:, kt, hp, 0:65],
                            pt[:, 0, col0:], start=(kt == 0), stop=(kt == nkt_c - 1),
                            skip_group_check=True)
                        if par_major and kt == nkt_c - 1:
                            # par0 sums done: 1/Z + broadcast overlap par1's AVs
                            with nc.allow_low_precision(reason="1/Z bf16"):
                                nc.vector.reciprocal(rr2[64:65, :],
                                                     pav[64:65, 0, :])
                            nc.vector.stream_shuffle(bc_sb[0:32, :],
                                                     rr2[64:96, :], [0] * 32)
                            nc.vector.stream_shuffle(bc_sb[32:64, :],
                                                     rr2[64:96, :], [0] * 32)
                    else:
                        nc.tensor.matmul(
                            pav[:, 1, col0:], v_sb[:, kt, hp, 64:192],
                            pt[:, 1, col0:], start=(kt == 0), stop=(kt == nkt_c - 1),
                            skip_group_check=True)
                # reciprocal of the sums rows + staging of unnormalized y to
                # SBUF: pav's slot frees as soon as these reads complete; the
                # broadcast + final scale run later behind other PE work
                with nc.allow_low_precision(reason="1/Z in bf16, ~0.4% rel"):
                    if not par_major:
                        nc.vector.reciprocal(rr2[64:65, :], pav[64:65, 0, :])
                    nc.vector.reciprocal(rr2[0:1, :], pav[0:1, 1, :])
                yc = rsmp.tile([128, CHUNK], BF16, tag="yc", name="yc")
                if c < 2 or c == NCH - 1:
                    nc.scalar.copy(yc[0:64, :], pav[0:64, 0, :])
                else:
                    nc.vector.tensor_copy(yc[0:64, :], pav[0:64, 0, :])
                nc.vector.tensor_copy(yc[64:128, :], pav[64:128, 1, :])
                return yc, rr2, bc_sb, par_major

            def emit_attn_hp(c, hp):
                return emit_attn_AV(c, hp, emit_attn_S(c, hp))

            def emit_norm(c, hp, yc, rr2, bc_sb, par_major, dump=False):
                # one K=2 ones matmul broadcasts both pars' 1/Z rows into a
                # full psum bank, then one full-width multiply against the
                # staged y (SBUF x PSUM - legal single-PSUM-operand form)
                cc = bass.ds(c * CHUNK, CHUNK)
                if not par_major:
                    nc.vector.stream_shuffle(bc_sb[0:32, :], rr2[64:96, :], [0] * 32)
                    nc.vector.stream_shuffle(bc_sb[32:64, :], rr2[64:96, :], [0] * 32)
                nc.vector.stream_shuffle(bc_sb[64:96, :], rr2[0:32, :], [0] * 32)
                nc.vector.stream_shuffle(bc_sb[96:128, :], rr2[0:32, :], [0] * 32)
                if c == NCH - 1 and hp == 1:
                    nc.vector.tensor_tensor(y_sb[:, hp, cc], yc[:], bc_sb[:], MULT)
                else:
                    nc.gpsimd.tensor_tensor(y_sb[:, hp, cc], yc[:], bc_sb[:], MULT)
                if dump:
                    nc.sync.dma_start(dbg_rr[:], rr2[:])
                    nc.sync.dma_start(dbg_bc[:], bc_sb[:])
                    nc.sync.dma_start(dbg_yc[:], yc[:])

            def emit_proj_last():
                # attention PSUM is free: all 8 output blocks get their own
                # bank; pt2=0 matmuls (needing only hp0's y) fill the PE while
                # hp1's softmax-normalize drains, pt2=1 + copies follow
                cc = bass.ds((NCH - 1) * CHUNK, CHUNK)
                prs = []
                for j in range(2):
                    prm = psmm.tile([128, CHUNK], F32, tag="mm", name="prm")
                    prs.append(prm[:])
                for j in range(2):
                    ps2 = pss.tile([128, 2, CHUNK], F32, tag="s", name=f"prs{j}")
                    prs += [ps2[:, 0, :], ps2[:, 1, :]]
                prv = psav.tile([128, 2, CHUNK], F32, tag="av", name="prv")
                prs += [prv[:, 0, :], prv[:, 1, :]]

                def phase_a():
                    for db, pr in enumerate(prs):
                        nc.tensor.matmul(
                            pr, wproj_sb[:, 0, db * 128:(db + 1) * 128],
                            y_sb[:, 0, cc], start=True, stop=False,
                            skip_group_check=True)

                def phase_b():
                    for db, pr in enumerate(prs):
                        nc.tensor.matmul(
                            pr, wproj_sb[:, 1, db * 128:(db + 1) * 128],
                            y_sb[:, 1, cc], start=False, stop=True,
                            skip_group_check=True)
                        o_sb = outsp.tile([128, CHUNK], BF16, tag="o")
                        if db % 2 == 0:
                            nc.scalar.copy(o_sb[:], pr)
                        else:
                            nc.vector.tensor_copy(o_sb[:], pr)
                        nc.sync.dma_start(yT[db * 128:(db + 1) * 128, cc], o_sb[:])
                return phase_a, phase_b

            def emit_proj(c):
                # pr slots alternate between the (idle-here) psav 2-bank slot
                # and psmm singles for a copy pipeline 4 deep
                cc = bass.ds(c * CHUNK, CHUNK)
                for quad in range(2):
                    prv = psav.tile([128, 2, CHUNK], F32, tag="av", name="prv")
                    prs = [prv[:, 0, :], prv[:, 1, :]]
                    for j in range(2):
                        prm = psmm.tile([128, CHUNK], F32, tag="mm", name="prm")
                        prs.append(prm[:])
                    for j, pr in enumerate(prs):
                        db = quad * 4 + j
                        for pt2 in range(2):
                            nc.tensor.matmul(
                                pr, wproj_sb[:, pt2, db * 128:(db + 1) * 128],
                                y_sb[:, pt2, cc], start=(pt2 == 0), stop=(pt2 == 1),
                                skip_group_check=True)
                        o_sb = outsp.tile([128, CHUNK], BF16, tag="o")
                        if db % 2 == 0 and c != 2:
                            nc.scalar.copy(o_sb[:], pr)
                        else:
                            nc.vector.tensor_copy(o_sb[:], pr)
                        nc.sync.dma_start(yT[db * 128:(db + 1) * 128, cc], o_sb[:])

            # software pipeline: next chunk's qkv matmuls sit between a head
            # pair's AV stream and its softmax-normalize so the PE never waits
            # on the reciprocal; proj(c-like work always trails norms.
            finish0 = emit_qkv0()
            emit_xload(1, slices=2)
            nc.sync.dma_start(bqk_sb[:], bqk[:])
            nc.sync.dma_start(cos_sb[:, 0:CHUNK], cos2[:, 0:CHUNK])
            nc.sync.dma_start(sin_sb[:, 0:CHUNK], sin2[:, 0:CHUNK])
            nc.sync.dma_start(tri_sb[:], trimask[:])
            nc.sync.dma_start(bv_sb[:], bv[:])
            nc.sync.dma_start(v_sb[:, :, :, 64:65], vones[:])
            emit_tail_dmas()
            finish0([0, 2], [0, 1, 2, 3])  # rope q01/k01 + all chunk-0 V
            emit_qkv_qk(1, [0, 1])      # PE filler while DVE ropes chunk 0
            for c in range(NCH):
                st0 = emit_attn_hp(c, 0) if c < NCH - 1 else None
                if c == 0:
                    finish0([1, 3], [])  # rope q23/k23 under hp0's S
                if c + 1 < NCH:
                    emit_xload(c + 2)
                    emit_qkv_qk(c + 1, [2, 3] if c == 0 else [0, 2])
                if c + 1 < NCH:
                    emit_norm(c, 0, *st0)
                    st1 = emit_attn_hp(c, 1)
                    if c > 0:
                        emit_qkv_qk(c + 1, [1, 3])
                    emit_qkv_v(c + 1)
                    emit_norm(c, 1, *st1)
                    if c + 1 == NCH - 1:
                        # feed the Act-bound last chunk early: its hp0 S/exp
                        # stream interleaves with this chunk's proj
                        p3_0 = emit_attn_S(c + 1, 0)
                    emit_proj(c)
                else:
                    # last chunk: hp0's normalize drains during hp1's attention
                    st0 = emit_attn_AV(c, 0, p3_0)
                    p3_1 = emit_attn_S(c, 1)
                    emit_norm(c, 0, *st0, dump=debug)
                    st1 = emit_attn_AV(c, 1, p3_1)
                    pa, pb = emit_proj_last()
                    pa()
                    emit_norm(c, 1, *st1)
                    pb()

            if debug:
                nc.sync.dma_start(dbg_qk[:], qk_sb[:])
                nc.sync.dma_start(dbg_v[:], v_sb[:])
                nc.sync.dma_start(dbg_y[:], y_sb[:].bitcast(F32))

    nc.finalize()
    return nc


def _host_inputs(x, Wqkv, bqkv, Wproj):
    """Per-core input maps. Core c: batch c//TP, heads [4*(c%TP), 4*(c%TP)+4)."""
    BF = ml_dtypes.bfloat16
    # RoPE tables in ^T layout, rows = head-local dim d (pattern repeats each 64)
    d = np.arange(64)
    inv_freq = 1.0 / (ROPE_BASE ** (np.arange(0, DH, 2, dtype=np.float64) / DH))  # [32]
    ang = np.arange(T, dtype=np.float64)[None, :] * inv_freq[d // 2][:, None]     # [64, T]
    cos64 = np.cos(ang)
    sin64 = np.sin(ang) * np.where(d % 2 == 0, -1.0, 1.0)[:, None]
    cos2 = np.tile(cos64, (2, 1)).astype(BF)
    sin2 = np.tile(sin64, (2, 1)).astype(np.float32)

    perm = np.zeros((128, 128), np.float32)
    perm[np.arange(128) ^ 1, np.arange(128)] = 1.0

    ki, qi = np.meshgrid(np.arange(128), np.arange(128), indexing="ij")
    tri = np.where(ki <= qi, 1.0, 0.0).astype(BF)
    tri3 = np.ascontiguousarray(np.broadcast_to(tri[:, None, :], (128, 2, 128)))

    Wq, Wk, Wv = Wqkv[:, :D], Wqkv[:, D:2 * D], Wqkv[:, 2 * D:]
    bq, bk, bvv = bqkv[:D], bqkv[D:2 * D], bqkv[2 * D:]

    maps = []
    for core in range(NCORES):
        b, r = core // TP, core % TP
        sl = slice(r * DIN, (r + 1) * DIN)
        wqk_c = np.concatenate([Wq[:, sl], Wk[:, sl]], axis=1)
        bqk_c = np.concatenate([bq[sl], bk[sl]]).astype(np.float32)
        maps.append({
            "xT": np.ascontiguousarray(x[b].T).astype(BF),
            "wqk": wqk_c.astype(BF),
            "wv": np.ascontiguousarray(Wv[:, sl]).astype(BF),
            "wproj": np.ascontiguousarray(Wproj[sl, :]).astype(BF),
            "bqk": np.ascontiguousarray(bqk_c.reshape(4, 128).T),
            "bv": np.broadcast_to(bvv[sl].astype(np.float32), (128, DIN)).copy(),
            "cos2": cos2,
            "sin2": sin2,
            "perm": perm.astype(BF),
            "trimask": tri3,
            "vones": np.ones((128, NKT, 2, 1), BF),
        })
    return maps


def kernel(x, Wqkv, bqkv, Wproj, bproj):
    global _compiled, _last_results
    from concourse.bass_utils import run_bass_kernel_spmd

    if _compiled is None:
        _compiled = _build()
    nc = _compiled

    maps = _host_inputs(
        np.asarray(x, np.float32), np.asarray(Wqkv, np.float32),
        np.asarray(bqkv, np.float32), np.asarray(Wproj, np.float32))
    res = run_bass_kernel_spmd(nc, maps, core_ids=list(range(NCORES)))
    _last_results = res
    out = np.empty((B, T, D), np.float32)
    for b in range(B):
        acc = np.zeros((D, T), np.float64)
        for r in range(TP):
            acc += np.asarray(res.results[b * TP + r]["yT"], dtype=np.float64)
        out[b] = acc.T + np.asarray(bproj, np.float64)[None, :]
    return out



# revision 53
# speedup vs baseline: 1.1336x; 1.1336x over previous
"""Causal self-attention (B=2, T=2048, D=1024, H=16, DH=64) on 8 trn2 cores.

Sharding: DP on batch (2) x TP on heads (4 heads/core). Each core computes
qkv for its heads from x[b]^T, RoPE, causal SDPA, and a partial row-parallel
output projection y^T [D, T]. Host sums TP partials, transposes, adds bias.

Inputs (x, Wqkv, Wv, Wproj) and the whole attention stage run in bf16, which
keeps the PE at 1 cycle/row even for narrow diagonal tiles, halves DMA
traffic, and unlocks DVE 16-bit modes; PSUM accumulation stays fp32 and the
final rel-err is ~5e-3 against the fp32 reference. Per (q-chunk, head-pair)
the even/odd-head score tiles land in one 2-bank PSUM tile so a single
activation instruction computes exp for both heads (PSUM accumulation groups
must never share a bank). The softmax denominator rides the AV matmul as an
extra ones-column shared by both heads: V is stored [pad|v_even|ones|v_odd]
per (k-tile, head-pair) so the par0 matmul puts Z_e at psum partition 64 and
the par1 matmul (output partitions 63:128) puts Z_o at 63. 1/Z rows are
partition-broadcast on the (idle) gpsimd engine and the final scale runs as a
single bf16 DVE multiply. The V bias rides the softmax identity
(y+b*Z)/Z = y/Z + b, i.e. it is folded into bproj on the host, so the V psum
drain is a pure strided copy.

Engine balance: PE does matmuls + RoPE permutes; Act does exp and the
qkv bias-adds (per-partition bias AP); DVE does RoPE multiplies, reciprocals,
V drains and normalize multiplies; gpsimd does causal masks, 1/Z broadcasts
and the auxiliary DMA queue. Aux input DMAs (rope tables, biases, masks,
wproj) issue on the gpsimd SWDGE queue so their descriptor generation runs
parallel to the critical wqk/x/wv stream on the shared HWDGE; PE warmup
(pstate ramp) runs on a memset tile so it starts immediately instead of
waiting for the first DMA.

Chunk 0's qkv runs o-major so matmuls chase the startup DMA stream (paired
wqk/x/wv slices of the contraction dim; HWDGE descriptor generation at
~625ns/DMA is the startup pacer). Later chunks prefetch x and interleave
next-chunk qkv between a head pair's AV stream and its normalize so the
statically-scheduled PE stream never waits on softmax latency. The last
chunk's S/exp stream is hoisted into the previous chunk's phase (the tail is
activation-bound) and its projection is split into pt2 phases across all 8
PSUM banks so output copies/DMAs drain while hp1's normalize completes; its
output DMAs alternate between the sync (HWDGE) and gpsimd (SWDGE) queues to
halve tail descriptor-generation serialization.
"""
import sys

if "/opt/trn_rl_repo" not in sys.path:
    sys.path.insert(0, "/opt/trn_rl_repo")

import numpy as np
import ml_dtypes

B, T, D = 2, 2048, 1024
H, DH = 16, 64
ROPE_BASE = 10000.0
NCORES = 8
TP = 4                # TP group size (cores per batch)
HL = H // TP          # heads per core = 4
CHUNK = 512           # t/q chunk
NCH = T // CHUNK      # 4
KT = 128              # k tile
NKT = T // KT         # 16
DIN = HL * DH         # 256 local head dims
SCALE = 1.0 / float(np.sqrt(DH))
NWARM = 36            # PE warmup matmuls (pstate ramp)

_compiled = None
_last_results = None


def _build(debug=False):
    import concourse.bass as bass
    import concourse.mybir as mybir
    import concourse.tile as tile
    from concourse import bacc

    F32 = mybir.dt.float32
    BF16 = mybir.dt.bfloat16
    ADD = mybir.AluOpType.add
    MULT = mybir.AluOpType.mult
    EXP = mybir.ActivationFunctionType.Exp
    IDENT = mybir.ActivationFunctionType.Identity

    nc = bacc.Bacc("TRN2", target_bir_lowering=False, num_devices=NCORES)

    xT = nc.dram_tensor("xT", [D, T], BF16, kind="ExternalInput")
    wqk = nc.dram_tensor("wqk", [D, 2 * DIN], BF16, kind="ExternalInput")
    wv = nc.dram_tensor("wv", [D, DIN], BF16, kind="ExternalInput")
    wproj = nc.dram_tensor("wproj", [DIN, D], BF16, kind="ExternalInput")
    bqk = nc.dram_tensor("bqk", [128, 4], F32, kind="ExternalInput")
    cos2 = nc.dram_tensor("cos2", [128, T], BF16, kind="ExternalInput")
    sin2 = nc.dram_tensor("sin2", [128, T], F32, kind="ExternalInput")
    perm = nc.dram_tensor("perm", [128, 128], BF16, kind="ExternalInput")
    trimask = nc.dram_tensor("trimask", [128, 2, 128], BF16, kind="ExternalInput")
    yT = nc.dram_tensor("yT", [D, T], BF16, kind="ExternalOutput")
    if debug:
        dbg_qk = nc.dram_tensor("dbg_qk", [128, 4, T], BF16, kind="ExternalOutput")
        dbg_v = nc.dram_tensor("dbg_v", [128, NKT, 2, 192], BF16, kind="ExternalOutput")
        dbg_y = nc.dram_tensor("dbg_y", [128, 2, T], F32, kind="ExternalOutput")
        dbg_rr = nc.dram_tensor("dbg_rr", [96, CHUNK], F32, kind="ExternalOutput")
        dbg_bc = nc.dram_tensor("dbg_bc", [128, CHUNK], F32, kind="ExternalOutput")
        dbg_yc = nc.dram_tensor("dbg_yc", [128, CHUNK], F32, kind="ExternalOutput")

    xT3 = xT[:].rearrange("(o p) t -> p o t", p=128)
    wqk3 = wqk[:].rearrange("(o p) f -> p o f", p=128)
    wv3 = wv[:].rearrange("(o p) f -> p o f", p=128)

    with tile.TileContext(nc) as tc:
        with tc.tile_pool(name="const", bufs=1) as constp, \
             tc.tile_pool(name="big", bufs=1) as bigp, \
             tc.tile_pool(name="xin", bufs=3) as xinp, \
             tc.tile_pool(name="ptile", bufs=36) as ptp, \
             tc.tile_pool(name="tmp", bufs=5) as tmpp, \
             tc.tile_pool(name="rsm", bufs=6) as rsmp, \
             tc.tile_pool(name="outs", bufs=8) as outsp, \
             tc.tile_pool(name="psmm", bufs=2, space="PSUM") as psmm, \
             tc.tile_pool(name="pss", bufs=2, space="PSUM") as pss, \
             tc.tile_pool(name="psav", bufs=2, space="PSUM") as psav:

            # ---- persistent SBUF tensors ----
            warm_sb = constp.tile([128, 128], BF16)
            wqk_sb = constp.tile([128, 8, 2 * DIN], BF16)     # [p, din_o, f]
            wv_sb = constp.tile([128, 8, DIN], BF16)
            wproj_sb = constp.tile([128, 2, D], BF16)         # [p, din_tile, dout]
            bqk_sb = constp.tile([128, 4], F32)
            cos_sb = constp.tile([128, T], BF16)
            sin_sb = constp.tile([128, T], F32)
            perm_sb = constp.tile([128, 128], BF16)
            tri_sb = constp.tile([128, 2, 128], BF16)

            qk_sb = bigp.tile([128, 4, T], BF16)              # fb: q01,q23,k01,k23
            # per (kt, hp): [v_even(0:64), ones(64), pad(65:128), v_odd
            # (128:192)]. The shared ones column gives Z_e at psum p64 (par0
            # = cols 0:65 -> p0:65) and Z_o at p0 (par1 = cols 64:192 ->
            # p0:128; the uninitialized pad cols feed junk partitions 1:64
            # that are never read). PSUM matmul outputs must start at
            # partition 0 or 64 (a base-32 start is limited to 32 rows).
            v_sb = bigp.tile([128, NKT, 2, 192], BF16)
            y_sb = bigp.tile([128, 2, T], BF16)               # y^T (din on partitions)

            # PE pstate warmup: dep-free matmul chain starts ~150ns in. The
            # init memset is emitted as a raw InstMemset pinned to DVE --
            # nc.*.memset lowers to an any-engine op that the scheduler puts
            # on Pool BEHIND the framework's preamble memsets (~1us late).
            nc.vector.add_instruction(mybir.InstMemset(
                name=nc.get_next_instruction_name(),
                mode="Const", constant=15872,  # bf16 0.125
                ins=[], outs=[nc.vector.lower_ap(warm_sb[:])]))
            pwm = psmm.tile([128, 128], F32, tag="mm", name="pwm")
            for _ in range(NWARM):
                nc.tensor.matmul(pwm[:], warm_sb[:], warm_sb[:], start=True,
                                 stop=True)

            # startup is a 2-stream DMA race: the small rope/bias/mask tables
            # go first on the gpsimd SWDGE queue; the big wqk/x/wv stream
            # feeds chunk 0's o-major qkv from the HWDGE queue in
            # consumption order
            nc.gpsimd.dma_start(perm_sb[:], perm[:])
            nc.gpsimd.dma_start(bqk_sb[:], bqk[:])
            nc.gpsimd.dma_start(cos_sb[:, 0:CHUNK], cos2[:, 0:CHUNK])
            nc.gpsimd.dma_start(sin_sb[:, 0:CHUNK], sin2[:, 0:CHUNK])
            nc.gpsimd.dma_start(tri_sb[:], trimask[:])
            nc.vector.memset(v_sb[:, :, :, 64:65], 1.0)  # softmax-Z ones col
            x_tiles = {}
            x_tiles[0] = xinp.tile([128, 8, CHUNK], BF16, tag="xchunk", name="x_c0")
            for o2 in range(4):
                o = bass.ds(2 * o2, 2)
                nc.sync.dma_start(wqk_sb[:, o], wqk3[:, o])
                nc.sync.dma_start(x_tiles[0][:, o], xT3[:, o, 0:CHUNK])
            for h in range(2):
                nc.sync.dma_start(wv_sb[:, bass.ds(4 * h, 4)],
                                  wv3[:, bass.ds(4 * h, 4)])

            def emit_tail_dmas():
                # late-use aux loads, still on the SWDGE queue
                nc.gpsimd.dma_start(cos_sb[:, CHUNK:], cos2[:, CHUNK:])
                nc.gpsimd.dma_start(sin_sb[:, CHUNK:], sin2[:, CHUNK:])
                nc.gpsimd.dma_start(wproj_sb[:],
                                    wproj[:].rearrange("(o p) f -> p o f", p=128))

            def emit_rope(c, fb, qkpre):
                # roped = qkpre*cos + perm(qkpre)*sin, written to qk_sb
                cc = bass.ds(c * CHUNK, CHUNK)
                pp = psmm.tile([128, CHUNK], F32, tag="mm", name="pp")
                nc.tensor.matmul(pp[:], perm_sb[:], qkpre[:], start=True, stop=True)
                nc.vector.tensor_tensor(qk_sb[:, fb, cc], qkpre[:], cos_sb[:, cc], MULT)
                swapped = tmpp.tile([128, CHUNK], BF16, tag="rope")
                nc.vector.tensor_tensor(swapped[:], pp[:], sin_sb[:, cc], MULT)
                nc.vector.tensor_tensor(qk_sb[:, fb, cc], qk_sb[:, fb, cc], swapped[:], ADD)

            # ropes are deferred one step behind their bias drains so the
            # perm matmul never sits on the PE right after its own pq stop
            # (the bias latency would stall the PE); flushed at the start of
            # the next PE-heavy block
            pending_ropes = []

            def flush_ropes():
                while pending_ropes:
                    emit_rope(*pending_ropes.pop(0))

            def emit_bias(qkpre, pq, fb, act):
                # psum->sbuf drain with the qk bias riding it. Act while its
                # exp stream is still light (early chunks), DVE afterwards.
                if act:
                    nc.scalar.activation(qkpre[:], pq, IDENT,
                                         bias=bqk_sb[:, fb:fb + 1], scale=1.0)
                else:
                    nc.vector.tensor_scalar_add(qkpre[:], pq,
                                                bqk_sb[:, fb:fb + 1])

            def emit_vwrite(kt, hp, pv):
                # pure strided copy (V bias is folded into bproj on the host):
                # pv cols [hp*128, hp*128+128) -> v_sb cols {0:64, 128:192}
                vdst = v_sb[:, kt, hp].rearrange("p (a b) -> p a b", b=64)[:, 0::2]
                vsrc = pv[:, hp * 128:(hp + 1) * 128].rearrange("p (a b) -> p a b", b=64)
                nc.vector.tensor_copy(vdst, vsrc)

            def emit_qkv0():
                # chunk 0: o-major so matmuls chase the startup DMA stream
                x_sb = x_tiles.pop(0)
                # borrow the attention-stage pss slots (idle during startup):
                # two 2-bank slots hold the four q/k blocks, one per bank
                # (PSUM accumulation groups must not share a bank)
                psq = [pss.tile([128, 2, CHUNK], F32, tag="s", name=f"psq{i}")
                       for i in range(2)]
                pqs = [psq[fb // 2][:, fb % 2, :] for fb in range(4)]
                for o in range(8):
                    for fb in range(4):
                        nc.tensor.matmul(
                            pqs[fb], wqk_sb[:, o, fb * 128:(fb + 1) * 128],
                            x_sb[:, o], start=(o == 0), stop=(o == 7),
                            skip_group_check=True)
                def finish(fbs, tbs):
                    for fb in fbs:
                        qkpre = tmpp.tile([128, CHUNK], BF16, tag="qkpre")
                        emit_bias(qkpre, pqs[fb], fb, act=True)
                        emit_rope(0, fb, qkpre)
                    for tb in tbs:
                        pvfull = psmm.tile([128, CHUNK], F32, tag="mm",
                                           name="pvfull")
                        pv = pvfull[:, :DIN]
                        for o in range(8):
                            nc.tensor.matmul(
                                pv[:], x_sb[:, o, tb * 128:(tb + 1) * 128],
                                wv_sb[:, o], start=(o == 0), stop=(o == 7))
                        for hp in range(2):
                            emit_vwrite(tb, hp, pv[:])
                return finish

            def emit_qkv_qk(c, fbs=range(4)):
                # q^T,k^T for chunk c: [f, t], bias-add + deferred rope
                x_sb = x_tiles[c]
                for fb in fbs:
                    pq = psmm.tile([128, CHUNK], F32, tag="mm", name="pq")
                    for o in range(8):
                        nc.tensor.matmul(
                            pq[:], wqk_sb[:, o, fb * 128:(fb + 1) * 128], x_sb[:, o],
                            start=(o == 0), stop=(o == 7))
                    qkpre = tmpp.tile([128, CHUNK], BF16, tag="qkpre")
                    emit_bias(qkpre, pq[:], fb, act=(c <= 1))
                    pending_ropes.append((c, fb, qkpre))
                    if len(pending_ropes) > 1:
                        emit_rope(*pending_ropes.pop(0))

            def emit_qkv_v(c):
                x_sb = x_tiles.pop(c)
                for tb in range(4):
                    if tb == 1:
                        flush_ropes()
                    kt = c * 4 + tb
                    pvfull = psmm.tile([128, CHUNK], F32, tag="mm", name="pvfull")
                    pv = pvfull[:, :DIN]
                    for o in range(8):
                        nc.tensor.matmul(
                            pv[:], x_sb[:, o, tb * 128:(tb + 1) * 128], wv_sb[:, o],
                            start=(o == 0), stop=(o == 7))
                    for hp in range(2):
                        emit_vwrite(kt, hp, pv[:])

            def emit_xload(c, slices=1):
                if c < NCH:
                    x_tiles[c] = xinp.tile([128, 8, CHUNK], BF16, tag="xchunk",
                                           name=f"x_c{c}")
                    cc = bass.ds(c * CHUNK, CHUNK)
                    w = 8 // slices
                    for i in range(slices):
                        nc.sync.dma_start(x_tiles[c][:, i * w:(i + 1) * w],
                                          xT3[:, i * w:(i + 1) * w, cc])

            def emit_attn_S(c, hp):
                # S + exp + mask for q-chunk c, head pair hp; even head uses
                # PE rows 0-63 / psum par 0, odd head rows 64-127 / par 1.
                nkt_c = 4 * c + 4
                p_tiles = []
                for kt in range(nkt_c):
                    if kt == 1:
                        flush_ropes()
                    i = kt - 4 * c  # >=0 on diagonal tiles
                    col0 = 128 * i if i >= 0 else 0
                    ps = pss.tile([128, 2, CHUNK], F32, tag="s", name="ps")
                    for par in range(2):
                        base = 64 * par
                        nc.tensor.matmul(
                            ps[:, par, col0:],
                            qk_sb[base:base + 64, 2 + hp, kt * 128:(kt + 1) * 128],
                            qk_sb[base:base + 64, hp, bass.ds(c * CHUNK + col0,
                                                              CHUNK - col0)],
                            start=True, stop=True, skip_group_check=True)
                    pt = ptp.tile([128, 2, CHUNK], BF16, tag="p", name="pt")
                    nc.scalar.activation(pt[:, :, col0:], ps[:, :, col0:], EXP,
                                         bias=0.0, scale=SCALE)
                    if i >= 0:
                        # zero k>q entries of the diagonal block (0/1 mask);
                        # cheap on DVE in bf16 2x mode (~200ns vs ~600 Pool)
                        nc.vector.tensor_tensor(
                            pt[:, :, col0:col0 + 128], pt[:, :, col0:col0 + 128],
                            tri_sb[:], MULT)
                    p_tiles.append(pt)
                return p_tiles

            def emit_attn_AV(c, hp, p_tiles):
                # par-MAJOR: all of par0's AV accumulation first, so Z_e's
                # reciprocal + broadcast + y_e staging overlap the par1 matmul
                # stream; only par1's short chain is exposed at the end.
                nkt_c = 4 * c + 4
                last = c == NCH - 1
                pav0 = psav.tile([128, CHUNK], F32, tag="av", name="pav0")
                pav1 = psav.tile([128, CHUNK], F32, tag="av", name="pav1")
                rr2 = rsmp.tile([96, CHUNK], BF16, tag="r", name="rr2")
                yc = rsmp.tile([128, CHUNK], BF16, tag="yc", name="yc")
                bc_sb = rsmp.tile([128, CHUNK], BF16, tag="bc", name="bc_sb")

                def stream(par):
                    for kt in range(nkt_c):
                        i = kt - 4 * c
                        col0 = 128 * i if i >= 0 else 0
                        pt = p_tiles[kt]
                        if par == 0:
                            nc.tensor.matmul(
                                pav0[:65, col0:], v_sb[:, kt, hp, 0:65],
                                pt[:, 0, col0:], start=(kt == 0),
                                stop=(kt == nkt_c - 1), skip_group_check=True)
                        else:
                            nc.tensor.matmul(
                                pav1[:, col0:], v_sb[:, kt, hp, 64:192],
                                pt[:, 1, col0:], start=(kt == 0),
                                stop=(kt == nkt_c - 1), skip_group_check=True)

                stream(0)
                with nc.allow_low_precision(reason="1/Z in bf16, ~0.4% rel"):
                    nc.vector.reciprocal(rr2[64:65, :], pav0[64:65, :])
                # 1/Z broadcast = 0-stride-source SBUF->SBUF DMA: no engine
                # cost, ~2.4us latency hidden under the par1 stream / the
                # deferred norm multiply
                nc.sync.dma_start(
                    bc_sb[0:64, :],
                    rr2[64:65, :].rearrange("p (o t) -> p o t", o=1)
                    .to_broadcast((1, 64, CHUNK)))
                nc.scalar.copy(yc[0:64, :], pav0[0:64, :])
                stream(1)
                with nc.allow_low_precision(reason="1/Z in bf16, ~0.4% rel"):
                    nc.vector.reciprocal(rr2[0:1, :], pav1[0:1, :])
                if last and hp == 1:
                    # chain-critical final normalize: stream_shuffle is lower
                    # latency than a DMA round trip
                    nc.vector.stream_shuffle(bc_sb[64:96, :], rr2[0:32, :], [0] * 32)
                    nc.vector.stream_shuffle(bc_sb[96:128, :], rr2[0:32, :], [0] * 32)
                else:
                    nc.sync.dma_start(
                        bc_sb[64:128, :],
                        rr2[0:1, :].rearrange("p (o t) -> p o t", o=1)
                        .to_broadcast((1, 64, CHUNK)))
                nc.scalar.copy(yc[64:128, :], pav1[64:128, :])
                return yc, bc_sb

            def emit_attn_hp(c, hp):
                return emit_attn_AV(c, hp, emit_attn_S(c, hp))

            def emit_norm(c, hp, yc, bc_sb, dump=False):
                # final softmax scale: one full-width bf16 multiply on DVE
                # (2x mode); emitted late so its bc wait never blocks earlier
                # DVE work on the in-order queue
                cc = bass.ds(c * CHUNK, CHUNK)
                nc.vector.tensor_tensor(y_sb[:, hp, cc], yc[:], bc_sb[:], MULT)
                if dump:
                    nc.sync.dma_start(dbg_bc[:], bc_sb[:].bitcast(F32)[:, :CHUNK // 2])
                    nc.sync.dma_start(dbg_yc[:], yc[:].bitcast(F32)[:, :CHUNK // 2])

            def emit_proj_last():
                # attention PSUM is free: all 8 output blocks get their own
                # bank; pt2=0 matmuls (needing only hp0's y) fill the PE while
                # hp1's softmax-normalize drains, pt2=1 + copies follow
                cc = bass.ds((NCH - 1) * CHUNK, CHUNK)
                prm0 = psmm.tile([128, CHUNK], F32, tag="mm", name="prm")
                prm1 = psmm.tile([128, CHUNK], F32, tag="mm", name="prm")
                ps20 = pss.tile([128, 2, CHUNK], F32, tag="s", name="prs0")
                ps21 = pss.tile([128, 2, CHUNK], F32, tag="s", name="prs1")
                prva = psav.tile([128, CHUNK], F32, tag="av", name="prva")
                prvb = psav.tile([128, CHUNK], F32, tag="av", name="prvb")
                # bank -> output block db is fixed; iteration orders differ:
                # phase_a defers the psav banks (blocked on pav(3,1) readers),
                # phase_b spaces a 2-bank tile's second stop 3+ matmuls after
                # its first half's copy (whole-tile WAR tracking would
                # otherwise stall the PE)
                banks = {0: prm0[:], 1: prm1[:], 2: ps20[:, 0, :],
                         3: ps20[:, 1, :], 4: ps21[:, 0, :], 5: ps21[:, 1, :],
                         6: prva[:], 7: prvb[:]}
                a_order = [0, 1, 2, 3, 4, 5, 6, 7]
                b_order = [0, 1, 2, 4, 6, 3, 5, 7]

                def phase_a():
                    for db in a_order:
                        nc.tensor.matmul(
                            banks[db], wproj_sb[:, 0, db * 128:(db + 1) * 128],
                            y_sb[:, 0, cc], start=True, stop=False,
                            skip_group_check=True)

                def phase_b():
                    for pos, db in enumerate(b_order):
                        nc.tensor.matmul(
                            banks[db], wproj_sb[:, 1, db * 128:(db + 1) * 128],
                            y_sb[:, 1, cc], start=False, stop=True,
                            skip_group_check=True)
                        o_sb = outsp.tile([128, CHUNK], BF16, tag="o")
                        # alternate copy engines and DGE queues by EMISSION
                        # position so the final copies/DMAs never serialize on
                        # one engine; the last bank gets the faster HWDGE path
                        if pos % 2 == 0:
                            nc.scalar.copy(o_sb[:], banks[db])
                        else:
                            nc.vector.tensor_copy(o_sb[:], banks[db])
                        eng = nc.gpsimd if pos % 2 == 0 else nc.sync
                        eng.dma_start(yT[db * 128:(db + 1) * 128, cc], o_sb[:])
                return phase_a, phase_b

            def emit_proj(c):
                # pr slots alternate between the (idle-here) psav 2-bank slot
                # and psmm singles; all 4 stops of a quad are issued before
                # its copies so whole-tile WAR tracking on the 2-bank psav
                # tile never stalls the PE
                cc = bass.ds(c * CHUNK, CHUNK)
                for quad in range(2):
                    prs = [psav.tile([128, CHUNK], F32, tag="av", name="prv")[:]
                           for _ in range(2)]
                    for j in range(2):
                        prm = psmm.tile([128, CHUNK], F32, tag="mm", name="prm")
                        prs.append(prm[:])
                    for j, pr in enumerate(prs):
                        db = quad * 4 + j
                        for pt2 in range(2):
                            nc.tensor.matmul(
                                pr, wproj_sb[:, pt2, db * 128:(db + 1) * 128],
                                y_sb[:, pt2, cc], start=(pt2 == 0), stop=(pt2 == 1),
                                skip_group_check=True)
                    for j, pr in enumerate(prs):
                        db = quad * 4 + j
                        o_sb = outsp.tile([128, CHUNK], BF16, tag="o")
                        if db % 2 == 0 and c != 2:
                            nc.scalar.copy(o_sb[:], pr)
                        else:
                            nc.vector.tensor_copy(o_sb[:], pr)
                        nc.sync.dma_start(yT[db * 128:(db + 1) * 128, cc], o_sb[:])

            # software pipeline: next chunk's qkv matmuls sit between a head
            # pair's AV stream and its softmax-normalize so the PE never waits
            # on the reciprocal; proj work always trails norms.
            finish0 = emit_qkv0()
            emit_xload(1, slices=2)
            emit_tail_dmas()
            finish0([0, 2], [0, 1, 2, 3])  # rope q01/k01 + all chunk-0 V
            emit_qkv_qk(1, [0, 1])      # PE filler while DVE ropes chunk 0
            for c in range(NCH):
                st0 = emit_attn_hp(c, 0) if c < NCH - 1 else None
                if c == 0:
                    finish0([1, 3], [])  # rope q23/k23 under hp0's S
                if c + 1 < NCH:
                    emit_xload(c + 2)
                    emit_qkv_qk(c + 1, [2, 3] if c == 0 else [0, 2])
                if c + 1 < NCH:
                    emit_norm(c, 0, *st0)
                    st1 = emit_attn_hp(c, 1)
                    if c > 0:
                        emit_qkv_qk(c + 1, [1, 3])
                    emit_qkv_v(c + 1)
                    emit_norm(c, 1, *st1)
                    if c + 1 == NCH - 1:
                        # feed the Act-bound last chunk early: its hp0 S/exp
                        # stream interleaves with this chunk's proj
                        p3_0 = emit_attn_S(c + 1, 0)
                    emit_proj(c)
                else:
                    # last chunk: hp0's normalize drains during hp1's
                    # attention; norm(3,0) before S(3,1) keeps its DVE chain
                    # ahead of the exp-paced masks
                    st0 = emit_attn_AV(c, 0, p3_0)
                    emit_norm(c, 0, *st0, dump=debug)
                    p3_1 = emit_attn_S(c, 1)
                    st1 = emit_attn_AV(c, 1, p3_1)
                    pa, pb = emit_proj_last()
                    pa()
                    emit_norm(c, 1, *st1)
                    pb()

            if debug:
                nc.sync.dma_start(dbg_qk[:], qk_sb[:])
                nc.sync.dma_start(dbg_v[:], v_sb[:])
                nc.sync.dma_start(dbg_y[:], y_sb[:].bitcast(F32))

    nc.finalize()
    return nc


def _host_inputs(x, Wqkv, bqkv, Wproj):
    """Per-core input maps. Core c: batch c//TP, heads [4*(c%TP), 4*(c%TP)+4)."""
    BF = ml_dtypes.bfloat16
    # RoPE tables in ^T layout, rows = head-local dim d (pattern repeats each 64)
    d = np.arange(64)
    inv_freq = 1.0 / (ROPE_BASE ** (np.arange(0, DH, 2, dtype=np.float64) / DH))  # [32]
    ang = np.arange(T, dtype=np.float64)[None, :] * inv_freq[d // 2][:, None]     # [64, T]
    cos64 = np.cos(ang)
    sin64 = np.sin(ang) * np.where(d % 2 == 0, -1.0, 1.0)[:, None]
    cos2 = np.tile(cos64, (2, 1)).astype(BF)
    sin2 = np.tile(sin64, (2, 1)).astype(np.float32)

    perm = np.zeros((128, 128), np.float32)
    perm[np.arange(128) ^ 1, np.arange(128)] = 1.0

    ki, qi = np.meshgrid(np.arange(128), np.arange(128), indexing="ij")
    tri = np.where(ki <= qi, 1.0, 0.0).astype(BF)
    tri3 = np.ascontiguousarray(np.broadcast_to(tri[:, None, :], (128, 2, 128)))

    Wq, Wk = Wqkv[:, :D], Wqkv[:, D:2 * D]
    Wv = Wqkv[:, 2 * D:]
    bq, bk = bqkv[:D], bqkv[D:2 * D]

    maps = []
    for core in range(NCORES):
        b, r = core // TP, core % TP
        sl = slice(r * DIN, (r + 1) * DIN)
        wqk_c = np.concatenate([Wq[:, sl], Wk[:, sl]], axis=1)
        bqk_c = np.concatenate([bq[sl], bk[sl]]).astype(np.float32)
        maps.append({
            "xT": np.ascontiguousarray(x[b].T).astype(BF),
            "wqk": wqk_c.astype(BF),
            "wv": np.ascontiguousarray(Wv[:, sl]).astype(BF),
            "wproj": np.ascontiguousarray(Wproj[sl, :]).astype(BF),
            "bqk": np.ascontiguousarray(bqk_c.reshape(4, 128).T),
            "cos2": cos2,
            "sin2": sin2,
            "perm": perm.astype(BF),
            "trimask": tri3,
        })
    return maps


def kernel(x, Wqkv, bqkv, Wproj, bproj):
    global _compiled, _last_results
    from concourse.bass_utils import run_bass_kernel_spmd

    if _compiled is None:
        _compiled = _build()
    nc = _compiled

    x = np.asarray(x, np.float32)
    Wqkv = np.asarray(Wqkv, np.float32)
    bqkv = np.asarray(bqkv, np.float32)
    Wproj = np.asarray(Wproj, np.float32)
    maps = _host_inputs(x, Wqkv, bqkv, Wproj)
    res = run_bass_kernel_spmd(nc, maps, core_ids=list(range(NCORES)))
    _last_results = res
    # V bias rides the softmax identity (y+b*Z)/Z = y/Z+b: fold bv@Wproj into
    # the output bias (exact for any bqkv)
    bv = np.asarray(bqkv, np.float64)[2 * D:]
    bproj_eff = np.asarray(bproj, np.float64) + bv @ np.asarray(Wproj, np.float64)
    out = np.empty((B, T, D), np.float32)
    for b in range(B):
        acc = np.zeros((D, T), np.float64)
        for r in range(TP):
            acc += np.asarray(res.results[b * TP + r]["yT"], dtype=np.float64)
        out[b] = acc.T + bproj_eff[None, :]
    return out


# revision 64
# speedup vs baseline: 1.1582x; 1.0217x over previous
"""Causal self-attention (B=2, T=2048, D=1024, H=16, DH=64) on 8 trn2 cores.

Sharding: DP on batch (2) x TP on heads (4 heads/core). Each core computes
qkv for its heads from x[b]^T, RoPE, causal SDPA, and a partial row-parallel
output projection y^T [D, T]. Host sums TP partials, transposes, adds bias.

Inputs (x, Wqkv, Wv, Wproj) and the whole attention stage run in bf16, which
keeps the PE at 1 cycle/row even for narrow diagonal tiles, halves DMA
traffic, and unlocks DVE 16-bit modes; PSUM accumulation stays fp32 and the
final rel-err is ~5e-3 against the fp32 reference. Per (q-chunk, head-pair)
the even/odd-head score tiles land in one 2-bank PSUM tile so a single
activation instruction computes exp for both heads (PSUM accumulation groups
must never share a bank). The softmax denominator rides the AV matmul as an
extra ones-column shared by both heads: V is stored [pad|v_even|ones|v_odd]
per (k-tile, head-pair) so the par0 matmul puts Z_e at psum partition 64 and
the par1 matmul (output partitions 63:128) puts Z_o at 63. 1/Z rows are
partition-broadcast on the (idle) gpsimd engine and the final scale runs as a
single bf16 DVE multiply. The V bias rides the softmax identity
(y+b*Z)/Z = y/Z + b, i.e. it is folded into bproj on the host, so the V psum
drain is a pure strided copy.

Engine balance: PE does matmuls + RoPE permutes; Act does exp and the
qkv bias-adds (per-partition bias AP); DVE does RoPE multiplies, reciprocals,
V drains and normalize multiplies; gpsimd does causal masks, 1/Z broadcasts
and the auxiliary DMA queue. Aux input DMAs (rope tables, biases, masks,
wproj) issue on the gpsimd SWDGE queue so their descriptor generation runs
parallel to the critical wqk/x/wv stream on the shared HWDGE; PE warmup
(pstate ramp) runs on a memset tile so it starts immediately instead of
waiting for the first DMA.

Chunk 0's qkv runs o-major so matmuls chase the startup DMA stream (paired
wqk/x/wv slices of the contraction dim; HWDGE descriptor generation at
~625ns/DMA is the startup pacer). Later chunks prefetch x and interleave
next-chunk qkv between a head pair's AV stream and its normalize so the
statically-scheduled PE stream never waits on softmax latency. The last
chunk's S/exp stream is hoisted into the previous chunk's phase (the tail is
activation-bound) and its projection is split into pt2 phases across all 8
PSUM banks so output copies/DMAs drain while hp1's normalize completes; its
output DMAs alternate between the sync (HWDGE) and gpsimd (SWDGE) queues to
halve tail descriptor-generation serialization.
"""
import sys

if "/opt/trn_rl_repo" not in sys.path:
    sys.path.insert(0, "/opt/trn_rl_repo")

import numpy as np
import ml_dtypes

B, T, D = 2, 2048, 1024
H, DH = 16, 64
ROPE_BASE = 10000.0
NCORES = 8
TP = 4                # TP group size (cores per batch)
HL = H // TP          # heads per core = 4
CHUNK = 512           # t/q chunk
NCH = T // CHUNK      # 4
KT = 128              # k tile
NKT = T // KT         # 16
DIN = HL * DH         # 256 local head dims
SCALE = 1.0 / float(np.sqrt(DH))
NWARM = 36            # PE warmup matmuls (pstate ramp)

_compiled = None
_last_results = None


def _build(debug=False):
    import concourse.bass as bass
    import concourse.mybir as mybir
    import concourse.tile as tile
    from concourse import bacc

    F32 = mybir.dt.float32
    BF16 = mybir.dt.bfloat16
    ADD = mybir.AluOpType.add
    MULT = mybir.AluOpType.mult
    EXP = mybir.ActivationFunctionType.Exp
    IDENT = mybir.ActivationFunctionType.Identity

    nc = bacc.Bacc("TRN2", target_bir_lowering=False, num_devices=NCORES)

    xT = nc.dram_tensor("xT", [D, T], BF16, kind="ExternalInput")
    wqk = nc.dram_tensor("wqk", [D, 2 * DIN], BF16, kind="ExternalInput")
    wv = nc.dram_tensor("wv", [D, DIN], BF16, kind="ExternalInput")
    wproj = nc.dram_tensor("wproj", [DIN, D], BF16, kind="ExternalInput")
    bqk = nc.dram_tensor("bqk", [128, 4], F32, kind="ExternalInput")
    cos2 = nc.dram_tensor("cos2", [128, T], BF16, kind="ExternalInput")
    sin2 = nc.dram_tensor("sin2", [128, T], F32, kind="ExternalInput")
    perm = nc.dram_tensor("perm", [128, 128], BF16, kind="ExternalInput")
    trimask = nc.dram_tensor("trimask", [128, 2, 128], BF16, kind="ExternalInput")
    yT = nc.dram_tensor("yT", [D, T], BF16, kind="ExternalOutput")
    if debug:
        dbg_qk = nc.dram_tensor("dbg_qk", [128, 4, T], BF16, kind="ExternalOutput")
        dbg_v = nc.dram_tensor("dbg_v", [128, NKT, 2, 192], BF16, kind="ExternalOutput")
        dbg_y = nc.dram_tensor("dbg_y", [128, 2, T], F32, kind="ExternalOutput")
        dbg_rr = nc.dram_tensor("dbg_rr", [96, CHUNK], F32, kind="ExternalOutput")
        dbg_bc = nc.dram_tensor("dbg_bc", [128, CHUNK], F32, kind="ExternalOutput")
        dbg_yc = nc.dram_tensor("dbg_yc", [128, CHUNK], F32, kind="ExternalOutput")

    xT3 = xT[:].rearrange("(o p) t -> p o t", p=128)
    wqk3 = wqk[:].rearrange("(o p) f -> p o f", p=128)
    wv3 = wv[:].rearrange("(o p) f -> p o f", p=128)

    with tile.TileContext(nc) as tc:
        with tc.tile_pool(name="const", bufs=1) as constp, \
             tc.tile_pool(name="big", bufs=1) as bigp, \
             tc.tile_pool(name="xin", bufs=3) as xinp, \
             tc.tile_pool(name="ptile", bufs=36) as ptp, \
             tc.tile_pool(name="tmp", bufs=5) as tmpp, \
             tc.tile_pool(name="rsm", bufs=6) as rsmp, \
             tc.tile_pool(name="outs", bufs=8) as outsp, \
             tc.tile_pool(name="psmm", bufs=2, space="PSUM") as psmm, \
             tc.tile_pool(name="pss", bufs=2, space="PSUM") as pss, \
             tc.tile_pool(name="psav", bufs=2, space="PSUM") as psav:

            # ---- persistent SBUF tensors ----
            warm_sb = constp.tile([128, 128], BF16)
            wqk_sb = constp.tile([128, 8, 2 * DIN], BF16)     # [p, din_o, f]
            wv_sb = constp.tile([128, 8, DIN], BF16)
            wproj_sb = constp.tile([128, 2, D], BF16)         # [p, din_tile, dout]
            bqk_sb = constp.tile([128, 4], F32)
            cos_sb = constp.tile([128, T], BF16)
            sin_sb = constp.tile([128, T], F32)
            perm_sb = constp.tile([128, 128], BF16)
            tri_sb = constp.tile([128, 2, 128], BF16)

            qk_sb = bigp.tile([128, 4, T], BF16)              # fb: q01,q23,k01,k23
            # per (kt, hp): [v_even(0:64), ones(64), pad(65:128), v_odd
            # (128:192)]. The shared ones column gives Z_e at psum p64 (par0
            # = cols 0:65 -> p0:65) and Z_o at p0 (par1 = cols 64:192 ->
            # p0:128; the uninitialized pad cols feed junk partitions 1:64
            # that are never read). PSUM matmul outputs must start at
            # partition 0 or 64 (a base-32 start is limited to 32 rows).
            v_sb = bigp.tile([128, NKT, 2, 192], BF16)
            y_sb = bigp.tile([128, 2, T], BF16)               # y^T (din on partitions)

            # PE pstate warmup: dep-free matmul chain starts ~150ns in. The
            # init memset is emitted as a raw InstMemset pinned to DVE --
            # nc.*.memset lowers to an any-engine op that the scheduler puts
            # on Pool BEHIND the framework's preamble memsets (~1us late).
            nc.vector.add_instruction(mybir.InstMemset(
                name=nc.get_next_instruction_name(),
                mode="Const", constant=15872,  # bf16 0.125
                ins=[], outs=[nc.vector.lower_ap(warm_sb[:])]))
            pwm = psmm.tile([128, 128], F32, tag="mm", name="pwm")
            for _ in range(NWARM):
                nc.tensor.matmul(pwm[:], warm_sb[:], warm_sb[:], start=True,
                                 stop=True)

            # startup is a 2-stream DMA race: the small rope/bias/mask tables
            # go first on the gpsimd SWDGE queue; the big wqk/x/wv stream
            # feeds chunk 0's o-major qkv from the HWDGE queue in
            # consumption order
            nc.gpsimd.dma_start(perm_sb[:], perm[:])
            nc.gpsimd.dma_start(bqk_sb[:], bqk[:])
            nc.gpsimd.dma_start(cos_sb[:, 0:CHUNK], cos2[:, 0:CHUNK])
            nc.gpsimd.dma_start(sin_sb[:, 0:CHUNK], sin2[:, 0:CHUNK])
            nc.gpsimd.dma_start(tri_sb[:], trimask[:])
            nc.vector.memset(v_sb[:, :, :, 64:65], 1.0)  # softmax-Z ones col
            x_tiles = {}
            x_tiles[0] = xinp.tile([128, 8, CHUNK], BF16, tag="xchunk", name="x_c0")
            for o2 in range(4):
                o = bass.ds(2 * o2, 2)
                nc.sync.dma_start(wqk_sb[:, o], wqk3[:, o])
                nc.sync.dma_start(x_tiles[0][:, o], xT3[:, o, 0:CHUNK])
            for h in range(2):
                nc.sync.dma_start(wv_sb[:, bass.ds(4 * h, 4)],
                                  wv3[:, bass.ds(4 * h, 4)])

            def emit_tail_dmas():
                # late-use aux loads, still on the SWDGE queue
                nc.gpsimd.dma_start(cos_sb[:, CHUNK:], cos2[:, CHUNK:])
                nc.gpsimd.dma_start(sin_sb[:, CHUNK:], sin2[:, CHUNK:])
                nc.gpsimd.dma_start(wproj_sb[:],
                                    wproj[:].rearrange("(o p) f -> p o f", p=128))

            def emit_rope(c, fb, qkpre):
                # roped = qkpre*cos + perm(qkpre)*sin, written to qk_sb
                cc = bass.ds(c * CHUNK, CHUNK)
                pp = psmm.tile([128, CHUNK], F32, tag="mm", name="pp")
                nc.tensor.matmul(pp[:], perm_sb[:], qkpre[:], start=True, stop=True)
                nc.vector.tensor_tensor(qk_sb[:, fb, cc], qkpre[:], cos_sb[:, cc], MULT)
                swapped = tmpp.tile([128, CHUNK], BF16, tag="rope")
                nc.vector.tensor_tensor(swapped[:], pp[:], sin_sb[:, cc], MULT)
                nc.vector.tensor_tensor(qk_sb[:, fb, cc], qk_sb[:, fb, cc], swapped[:], ADD)

            # ropes are deferred one step behind their bias drains so the
            # perm matmul never sits on the PE right after its own pq stop
            # (the bias latency would stall the PE); flushed at the start of
            # the next PE-heavy block
            pending_ropes = []

            def flush_ropes():
                while pending_ropes:
                    emit_rope(*pending_ropes.pop(0))

            def emit_bias(qkpre, pq, fb, act):
                # psum->sbuf drain with the qk bias riding it. Act while its
                # exp stream is still light (early chunks), DVE afterwards.
                if act:
                    nc.scalar.activation(qkpre[:], pq, IDENT,
                                         bias=bqk_sb[:, fb:fb + 1], scale=1.0)
                else:
                    nc.vector.tensor_scalar_add(qkpre[:], pq,
                                                bqk_sb[:, fb:fb + 1])

            def emit_vwrite(kt, hp, pv):
                # pure strided copy (V bias is folded into bproj on the host):
                # pv cols [hp*128, hp*128+128) -> v_sb cols {0:64, 128:192}
                vdst = v_sb[:, kt, hp].rearrange("p (a b) -> p a b", b=64)[:, 0::2]
                vsrc = pv[:, hp * 128:(hp + 1) * 128].rearrange("p (a b) -> p a b", b=64)
                nc.vector.tensor_copy(vdst, vsrc)

            def emit_qkv0():
                # chunk 0: o-major so matmuls chase the startup DMA stream
                x_sb = x_tiles.pop(0)
                # borrow the attention-stage pss slots (idle during startup):
                # two 2-bank slots hold the four q/k blocks, one per bank
                # (PSUM accumulation groups must not share a bank)
                psq = [pss.tile([128, 2, CHUNK], F32, tag="s", name=f"psq{i}")
                       for i in range(2)]
                pqs = [psq[fb // 2][:, fb % 2, :] for fb in range(4)]
                for o in range(8):
                    for fb in range(4):
                        nc.tensor.matmul(
                            pqs[fb], wqk_sb[:, o, fb * 128:(fb + 1) * 128],
                            x_sb[:, o], start=(o == 0), stop=(o == 7),
                            skip_group_check=True)
                def finish(fbs, tbs):
                    for fb in fbs:
                        qkpre = tmpp.tile([128, CHUNK], BF16, tag="qkpre")
                        emit_bias(qkpre, pqs[fb], fb, act=True)
                        emit_rope(0, fb, qkpre)
                    for tb in tbs:
                        pvfull = psmm.tile([128, CHUNK], F32, tag="mm",
                                           name="pvfull")
                        pv = pvfull[:, :DIN]
                        for o in range(8):
                            nc.tensor.matmul(
                                pv[:], x_sb[:, o, tb * 128:(tb + 1) * 128],
                                wv_sb[:, o], start=(o == 0), stop=(o == 7))
                        for hp in range(2):
                            emit_vwrite(tb, hp, pv[:])
                return finish

            def emit_qkv_qk(c, fbs=range(4)):
                # q^T,k^T for chunk c: [f, t], bias-add + deferred rope
                x_sb = x_tiles[c]
                for fb in fbs:
                    pq = psmm.tile([128, CHUNK], F32, tag="mm", name="pq")
                    for o in range(8):
                        nc.tensor.matmul(
                            pq[:], wqk_sb[:, o, fb * 128:(fb + 1) * 128], x_sb[:, o],
                            start=(o == 0), stop=(o == 7))
                    qkpre = tmpp.tile([128, CHUNK], BF16, tag="qkpre")
                    emit_bias(qkpre, pq[:], fb, act=(c == 0))
                    pending_ropes.append((c, fb, qkpre))
                    if len(pending_ropes) > 1:
                        emit_rope(*pending_ropes.pop(0))

            def emit_qkv_v(c):
                x_sb = x_tiles.pop(c)
                for tb in range(4):
                    if tb == 1:
                        flush_ropes()
                    kt = c * 4 + tb
                    pvfull = psmm.tile([128, CHUNK], F32, tag="mm", name="pvfull")
                    pv = pvfull[:, :DIN]
                    for o in range(8):
                        nc.tensor.matmul(
                            pv[:], x_sb[:, o, tb * 128:(tb + 1) * 128], wv_sb[:, o],
                            start=(o == 0), stop=(o == 7))
                    for hp in range(2):
                        emit_vwrite(kt, hp, pv[:])

            def emit_xload(c, slices=1):
                if c < NCH:
                    x_tiles[c] = xinp.tile([128, 8, CHUNK], BF16, tag="xchunk",
                                           name=f"x_c{c}")
                    cc = bass.ds(c * CHUNK, CHUNK)
                    w = 8 // slices
                    for i in range(slices):
                        nc.sync.dma_start(x_tiles[c][:, i * w:(i + 1) * w],
                                          xT3[:, i * w:(i + 1) * w, cc])

            def emit_attn_S(c, hp):
                # S + exp + mask for q-chunk c, head pair hp; even head uses
                # PE rows 0-63 / psum par 0, odd head rows 64-127 / par 1.
                nkt_c = 4 * c + 4
                p_tiles = []
                for kt in range(nkt_c):
                    if kt == 1:
                        flush_ropes()
                    i = kt - 4 * c  # >=0 on diagonal tiles
                    col0 = 128 * i if i >= 0 else 0
                    ps = pss.tile([128, 2, CHUNK], F32, tag="s", name="ps")
                    for par in range(2):
                        base = 64 * par
                        nc.tensor.matmul(
                            ps[:, par, col0:],
                            qk_sb[base:base + 64, 2 + hp, kt * 128:(kt + 1) * 128],
                            qk_sb[base:base + 64, hp, bass.ds(c * CHUNK + col0,
                                                              CHUNK - col0)],
                            start=True, stop=True, skip_group_check=True)
                    pt = ptp.tile([128, 2, CHUNK], BF16, tag="p", name="pt")
                    nc.scalar.activation(pt[:, :, col0:], ps[:, :, col0:], EXP,
                                         bias=0.0, scale=SCALE)
                    if i >= 0:
                        # zero k>q entries of the diagonal block (0/1 mask);
                        # cheap on DVE in bf16 2x mode (~200ns vs ~600 Pool)
                        nc.vector.tensor_tensor(
                            pt[:, :, col0:col0 + 128], pt[:, :, col0:col0 + 128],
                            tri_sb[:], MULT)
                    p_tiles.append(pt)
                return p_tiles

            def emit_attn_AV(c, hp, p_tiles):
                # par-MAJOR: all of par0's AV accumulation first, so Z_e's
                # reciprocal + broadcast + y_e staging overlap the par1 matmul
                # stream; only par1's short chain is exposed at the end.
                nkt_c = 4 * c + 4
                last = c == NCH - 1
                pav0 = psav.tile([128, CHUNK], F32, tag="av", name="pav0")
                pav1 = psav.tile([128, CHUNK], F32, tag="av", name="pav1")
                rr2 = rsmp.tile([96, CHUNK], BF16, tag="r", name="rr2")
                yc = rsmp.tile([128, CHUNK], BF16, tag="yc", name="yc")
                bc_sb = rsmp.tile([128, CHUNK], BF16, tag="bc", name="bc_sb")

                def stream(par):
                    for kt in range(nkt_c):
                        i = kt - 4 * c
                        col0 = 128 * i if i >= 0 else 0
                        pt = p_tiles[kt]
                        if par == 0:
                            nc.tensor.matmul(
                                pav0[:65, col0:], v_sb[:, kt, hp, 0:65],
                                pt[:, 0, col0:], start=(kt == 0),
                                stop=(kt == nkt_c - 1), skip_group_check=True)
                        else:
                            nc.tensor.matmul(
                                pav1[:, col0:], v_sb[:, kt, hp, 64:192],
                                pt[:, 1, col0:], start=(kt == 0),
                                stop=(kt == nkt_c - 1), skip_group_check=True)

                stream(0)
                with nc.allow_low_precision(reason="1/Z in bf16, ~0.4% rel"):
                    nc.vector.reciprocal(rr2[64:65, :], pav0[64:65, :])
                # 1/Z broadcast = 0-stride-source SBUF->SBUF DMA: no engine
                # cost, ~2.4us latency hidden under the par1 stream / the
                # deferred norm multiply
                nc.sync.dma_start(
                    bc_sb[0:64, :],
                    rr2[64:65, :].rearrange("p (o t) -> p o t", o=1)
                    .to_broadcast((1, 64, CHUNK)))
                nc.scalar.copy(yc[0:64, :], pav0[0:64, :])
                stream(1)
                with nc.allow_low_precision(reason="1/Z in bf16, ~0.4% rel"):
                    nc.vector.reciprocal(rr2[0:1, :], pav1[0:1, :])
                if last and hp == 1:
                    # chain-critical final normalize: stream_shuffle is lower
                    # latency than a DMA round trip
                    nc.vector.stream_shuffle(bc_sb[64:96, :], rr2[0:32, :], [0] * 32)
                    nc.vector.stream_shuffle(bc_sb[96:128, :], rr2[0:32, :], [0] * 32)
                else:
                    nc.sync.dma_start(
                        bc_sb[64:128, :],
                        rr2[0:1, :].rearrange("p (o t) -> p o t", o=1)
                        .to_broadcast((1, 64, CHUNK)))
                if last:
                    nc.scalar.copy(yc[64:128, :], pav1[64:128, :])
                else:
                    nc.vector.tensor_copy(yc[64:128, :], pav1[64:128, :])
                return yc, bc_sb

            def emit_attn_hp(c, hp):
                return emit_attn_AV(c, hp, emit_attn_S(c, hp))

            def emit_norm(c, hp, yc, bc_sb, dump=False):
                # final softmax scale: one full-width bf16 multiply on DVE
                # (2x mode); emitted late so its bc wait never blocks earlier
                # DVE work on the in-order queue
                cc = bass.ds(c * CHUNK, CHUNK)
                nc.vector.tensor_tensor(y_sb[:, hp, cc], yc[:], bc_sb[:], MULT)
                if dump:
                    nc.sync.dma_start(dbg_bc[:], bc_sb[:].bitcast(F32)[:, :CHUNK // 2])
                    nc.sync.dma_start(dbg_yc[:], yc[:].bitcast(F32)[:, :CHUNK // 2])

            def emit_proj_last():
                # attention PSUM is free: all 8 output blocks get their own
                # bank; pt2=0 matmuls (needing only hp0's y) fill the PE while
                # hp1's softmax-normalize drains, pt2=1 + copies follow
                cc = bass.ds((NCH - 1) * CHUNK, CHUNK)
                prm0 = psmm.tile([128, CHUNK], F32, tag="mm", name="prm")
                prm1 = psmm.tile([128, CHUNK], F32, tag="mm", name="prm")
                ps20 = pss.tile([128, 2, CHUNK], F32, tag="s", name="prs0")
                ps21 = pss.tile([128, 2, CHUNK], F32, tag="s", name="prs1")
                prva = psav.tile([128, CHUNK], F32, tag="av", name="prva")
                prvb = psav.tile([128, CHUNK], F32, tag="av", name="prvb")
                # bank -> output block db is fixed; iteration orders differ:
                # phase_a defers the psav banks (blocked on pav(3,1) readers),
                # phase_b spaces a 2-bank tile's second stop 3+ matmuls after
                # its first half's copy (whole-tile WAR tracking would
                # otherwise stall the PE)
                banks = {0: prm0[:], 1: prm1[:], 2: ps20[:, 0, :],
                         3: ps20[:, 1, :], 4: ps21[:, 0, :], 5: ps21[:, 1, :],
                         6: prva[:], 7: prvb[:]}
                a_order = [0, 1, 2, 3, 4, 5, 6, 7]
                b_order = [0, 1, 2, 4, 6, 3, 5, 7]

                def phase_a():
                    for db in a_order:
                        nc.tensor.matmul(
                            banks[db], wproj_sb[:, 0, db * 128:(db + 1) * 128],
                            y_sb[:, 0, cc], start=True, stop=False,
                            skip_group_check=True)

                def phase_b():
                    for pos, db in enumerate(b_order):
                        nc.tensor.matmul(
                            banks[db], wproj_sb[:, 1, db * 128:(db + 1) * 128],
                            y_sb[:, 1, cc], start=False, stop=True,
                            skip_group_check=True)
                        o_sb = outsp.tile([128, CHUNK], BF16, tag="o")
                        # alternate copy engines by EMISSION position so the
                        # final copies never serialize on one engine. Early
                        # banks go out on the slow SWDGE (Pool) queue, late
                        # banks on HWDGE (625 vs 1038ns gen): the LAST DMA's
                        # gen must not queue behind three 1038ns SWDGE gens.
                        if pos % 2 == 0:
                            nc.scalar.copy(o_sb[:], banks[db])
                        else:
                            nc.vector.tensor_copy(o_sb[:], banks[db])
                        eng = nc.gpsimd if pos < 3 else nc.sync
                        eng.dma_start(yT[db * 128:(db + 1) * 128, cc], o_sb[:])
                return phase_a, phase_b

            def emit_proj(c):
                # pr slots alternate between the (idle-here) psav 2-bank slot
                # and psmm singles; all 4 stops of a quad are issued before
                # its copies so whole-tile WAR tracking on the 2-bank psav
                # tile never stalls the PE
                cc = bass.ds(c * CHUNK, CHUNK)
                for quad in range(2):
                    prs = [psav.tile([128, CHUNK], F32, tag="av", name="prv")[:]
                           for _ in range(2)]
                    for j in range(2):
                        prm = psmm.tile([128, CHUNK], F32, tag="mm", name="prm")
                        prs.append(prm[:])
                    for j, pr in enumerate(prs):
                        db = quad * 4 + j
                        for pt2 in range(2):
                            nc.tensor.matmul(
                                pr, wproj_sb[:, pt2, db * 128:(db + 1) * 128],
                                y_sb[:, pt2, cc], start=(pt2 == 0), stop=(pt2 == 1),
                                skip_group_check=True)
                    for j, pr in enumerate(prs):
                        db = quad * 4 + j
                        o_sb = outsp.tile([128, CHUNK], BF16, tag="o")
                        if db % 2 == 0 and c != 2:
                            nc.scalar.copy(o_sb[:], pr)
                        else:
                            nc.vector.tensor_copy(o_sb[:], pr)
                        nc.sync.dma_start(yT[db * 128:(db + 1) * 128, cc], o_sb[:])

            # software pipeline: next chunk's qkv matmuls sit between a head
            # pair's AV stream and its softmax-normalize so the PE never waits
            # on the reciprocal; proj work always trails norms.
            finish0 = emit_qkv0()
            emit_xload(1, slices=2)
            emit_tail_dmas()
            finish0([0, 2], [0, 1, 2, 3])  # rope q01/k01 + all chunk-0 V
            emit_qkv_qk(1, [0, 1])      # PE filler while DVE ropes chunk 0
            for c in range(NCH):
                st0 = emit_attn_hp(c, 0) if c < NCH - 1 else None
                if c == 0:
                    finish0([1, 3], [])  # rope q23/k23 under hp0's S
                if c + 1 < NCH:
                    emit_xload(c + 2)
                    emit_qkv_qk(c + 1, [2, 3] if c == 0 else [0, 2])
                if c + 1 < NCH:
                    emit_norm(c, 0, *st0)
                    st1 = emit_attn_hp(c, 1)
                    if c > 0:
                        emit_qkv_qk(c + 1, [1, 3])
                    emit_qkv_v(c + 1)
                    emit_norm(c, 1, *st1)
                    if c + 1 == NCH - 1:
                        # feed the Act-bound last chunk early: its hp0 S/exp
                        # stream interleaves with this chunk's proj
                        p3_0 = emit_attn_S(c + 1, 0)
                    emit_proj(c)
                else:
                    # ---- fused last chunk ----
                    # The tail is Act(exp)-bound: S(3,1)'s 16 exps pace
                    # everything. Interleave AV(3,0)-par0 with S(3,1) so the
                    # exp stream starts ~6us earlier, then AV(3,0)-par1 with
                    # AV(3,1)-par0. Masks go to Pool and the hp0 chains to
                    # DVE so nothing queues behind the exp stream on Act.
                    nkt_c = 4 * c + 4
                    pav0_0 = psav.tile([128, CHUNK], F32, tag="av", name="pav0")
                    pav1_0 = psav.tile([128, CHUNK], F32, tag="av", name="pav1")
                    rr_0 = rsmp.tile([96, CHUNK], BF16, tag="r", name="rr2")
                    yc_0 = rsmp.tile([128, CHUNK], BF16, tag="yc", name="yc")
                    bc_0 = rsmp.tile([128, CHUNK], BF16, tag="bc", name="bc_sb")

                    def av_mm(hp, par, pav, p_tiles, kt):
                        i = kt - 4 * c
                        col0 = 128 * i if i >= 0 else 0
                        pt = p_tiles[kt]
                        if par == 0:
                            nc.tensor.matmul(
                                pav[:65, col0:], v_sb[:, kt, hp, 0:65],
                                pt[:, 0, col0:], start=(kt == 0),
                                stop=(kt == nkt_c - 1), skip_group_check=True)
                        else:
                            nc.tensor.matmul(
                                pav[:, col0:], v_sb[:, kt, hp, 64:192],
                                pt[:, 1, col0:], start=(kt == 0),
                                stop=(kt == nkt_c - 1), skip_group_check=True)

                    p3_1 = []
                    for kt in range(nkt_c):
                        av_mm(0, 0, pav0_0, p3_0, kt)
                        i = kt - 4 * c
                        col0 = 128 * i if i >= 0 else 0
                        ps = pss.tile([128, 2, CHUNK], F32, tag="s", name="ps")
                        for par in range(2):
                            base = 64 * par
                            nc.tensor.matmul(
                                ps[:, par, col0:],
                                qk_sb[base:base + 64, 3, kt * 128:(kt + 1) * 128],
                                qk_sb[base:base + 64, 1,
                                      bass.ds(c * CHUNK + col0, CHUNK - col0)],
                                start=True, stop=True, skip_group_check=True)
                        pt = ptp.tile([128, 2, CHUNK], BF16, tag="p", name="pt")
                        nc.scalar.activation(pt[:, :, col0:], ps[:, :, col0:],
                                             EXP, bias=0.0, scale=SCALE)
                        if i >= 0:
                            nc.gpsimd.tensor_tensor(
                                pt[:, :, col0:col0 + 128],
                                pt[:, :, col0:col0 + 128], tri_sb[:], MULT)
                        p3_1.append(pt)
                    with nc.allow_low_precision(reason="1/Z in bf16"):
                        nc.vector.reciprocal(rr_0[64:65, :], pav0_0[64:65, :])
                    # last-chunk bc halves ride the Pool SWDGE queue: a
                    # waiting sync-queue DMA would head-block SP.SEQ and
                    # delay the tail output DMAs queued behind it
                    nc.gpsimd.dma_start(
                        bc_0[0:64, :],
                        rr_0[64:65, :].rearrange("p (o t) -> p o t", o=1)
                        .to_broadcast((1, 64, CHUNK)))
                    nc.vector.tensor_copy(yc_0[0:64, :], pav0_0[0:64, :])

                    pav0_1 = psav.tile([128, CHUNK], F32, tag="av", name="pav0")
                    for kt in range(nkt_c):
                        av_mm(0, 1, pav1_0, p3_0, kt)
                        av_mm(1, 0, pav0_1, p3_1, kt)
                    with nc.allow_low_precision(reason="1/Z in bf16"):
                        nc.vector.reciprocal(rr_0[0:1, :], pav1_0[0:1, :])
                    nc.gpsimd.dma_start(
                        bc_0[64:128, :],
                        rr_0[0:1, :].rearrange("p (o t) -> p o t", o=1)
                        .to_broadcast((1, 64, CHUNK)))
                    nc.vector.tensor_copy(yc_0[64:128, :], pav1_0[64:128, :])
                    rr_1 = rsmp.tile([96, CHUNK], BF16, tag="r", name="rr2")
                    yc_1 = rsmp.tile([128, CHUNK], BF16, tag="yc", name="yc")
                    bc_1 = rsmp.tile([128, CHUNK], BF16, tag="bc", name="bc_sb")
                    with nc.allow_low_precision(reason="1/Z in bf16"):
                        nc.vector.reciprocal(rr_1[64:65, :], pav0_1[64:65, :])
                    nc.gpsimd.dma_start(
                        bc_1[0:64, :],
                        rr_1[64:65, :].rearrange("p (o t) -> p o t", o=1)
                        .to_broadcast((1, 64, CHUNK)))
                    nc.scalar.copy(yc_1[0:64, :], pav0_1[0:64, :])
                    emit_norm(c, 0, yc_0, bc_0, dump=debug)

                    pav1_1 = psav.tile([128, CHUNK], F32, tag="av", name="pav1")
                    for kt in range(nkt_c):
                        av_mm(1, 1, pav1_1, p3_1, kt)
                    with nc.allow_low_precision(reason="1/Z in bf16"):
                        nc.vector.reciprocal(rr_1[0:1, :], pav1_1[0:1, :])
                    nc.vector.stream_shuffle(bc_1[64:96, :], rr_1[0:32, :], [0] * 32)
                    nc.vector.stream_shuffle(bc_1[96:128, :], rr_1[0:32, :], [0] * 32)
                    nc.scalar.copy(yc_1[64:128, :], pav1_1[64:128, :])
                    pa, pb = emit_proj_last()
                    pa()
                    emit_norm(c, 1, yc_1, bc_1)
                    pb()

            if debug:
                nc.sync.dma_start(dbg_qk[:], qk_sb[:])
                nc.sync.dma_start(dbg_v[:], v_sb[:])
                nc.sync.dma_start(dbg_y[:], y_sb[:].bitcast(F32))

    nc.finalize()
    return nc


def _host_inputs(x, Wqkv, bqkv, Wproj):
    """Per-core input maps. Core c: batch c//TP, heads [4*(c%TP), 4*(c%TP)+4)."""
    BF = ml_dtypes.bfloat16
    # RoPE tables in ^T layout, rows = head-local dim d (pattern repeats each 64)
    d = np.arange(64)
    inv_freq = 1.0 / (ROPE_BASE ** (np.arange(0, DH, 2, dtype=np.float64) / DH))  # [32]
    ang = np.arange(T, dtype=np.float64)[None, :] * inv_freq[d // 2][:, None]     # [64, T]
    cos64 = np.cos(ang)
    sin64 = np.sin(ang) * np.where(d % 2 == 0, -1.0, 1.0)[:, None]
    cos2 = np.tile(cos64, (2, 1)).astype(BF)
    sin2 = np.tile(sin64, (2, 1)).astype(np.float32)

    perm = np.zeros((128, 128), np.float32)
    perm[np.arange(128) ^ 1, np.arange(128)] = 1.0

    ki, qi = np.meshgrid(np.arange(128), np.arange(128), indexing="ij")
    tri = np.where(ki <= qi, 1.0, 0.0).astype(BF)
    tri3 = np.ascontiguousarray(np.broadcast_to(tri[:, None, :], (128, 2, 128)))

    Wq, Wk = Wqkv[:, :D], Wqkv[:, D:2 * D]
    Wv = Wqkv[:, 2 * D:]
    bq, bk = bqkv[:D], bqkv[D:2 * D]

    maps = []
    for core in range(NCORES):
        b, r = core // TP, core % TP
        sl = slice(r * DIN, (r + 1) * DIN)
        wqk_c = np.concatenate([Wq[:, sl], Wk[:, sl]], axis=1)
        bqk_c = np.concatenate([bq[sl], bk[sl]]).astype(np.float32)
        maps.append({
            "xT": np.ascontiguousarray(x[b].T).astype(BF),
            "wqk": wqk_c.astype(BF),
            "wv": np.ascontiguousarray(Wv[:, sl]).astype(BF),
            "wproj": np.ascontiguousarray(Wproj[sl, :]).astype(BF),
            "bqk": np.ascontiguousarray(bqk_c.reshape(4, 128).T),
            "cos2": cos2,
            "sin2": sin2,
            "perm": perm.astype(BF),
            "trimask": tri3,
        })
    return maps


def kernel(x, Wqkv, bqkv, Wproj, bproj):
    global _compiled, _last_results
    from concourse.bass_utils import run_bass_kernel_spmd

    if _compiled is None:
        _compiled = _build()
    nc = _compiled

    x = np.asarray(x, np.float32)
    Wqkv = np.asarray(Wqkv, np.float32)
    bqkv = np.asarray(bqkv, np.float32)
    Wproj = np.asarray(Wproj, np.float32)
    maps = _host_inputs(x, Wqkv, bqkv, Wproj)
    res = run_bass_kernel_spmd(nc, maps, core_ids=list(range(NCORES)))
    _last_results = res
    # V bias rides the softmax identity (y+b*Z)/Z = y/Z+b: fold bv@Wproj into
    # the output bias (exact for any bqkv)
    bv = np.asarray(bqkv, np.float64)[2 * D:]
    bproj_eff = np.asarray(bproj, np.float64) + bv @ np.asarray(Wproj, np.float64)
    out = np.empty((B, T, D), np.float32)
    for b in range(B):
        acc = np.zeros((D, T), np.float64)
        for r in range(TP):
            acc += np.asarray(res.results[b * TP + r]["yT"], dtype=np.float64)
        out[b] = acc.T + bproj_eff[None, :]
    return out


# revision 75
# speedup vs baseline: 1.1645x; 1.0055x over previous
"""Causal self-attention (B=2, T=2048, D=1024, H=16, DH=64) on 8 trn2 cores.

Sharding: DP on batch (2) x TP on heads (4 heads/core). Each core computes
qkv for its heads from x[b]^T, RoPE, causal SDPA, and a partial row-parallel
output projection y^T [D, T]. Host sums TP partials, transposes, adds bias.

Inputs (x, Wqkv, Wv, Wproj) and the whole attention stage run in bf16, which
keeps the PE at 1 cycle/row even for narrow diagonal tiles, halves DMA
traffic, and unlocks DVE 16-bit modes; PSUM accumulation stays fp32 and the
final rel-err is ~5e-3 against the fp32 reference. Per (q-chunk, head-pair)
the even/odd-head score tiles land in one 2-bank PSUM tile so a single
activation instruction computes exp for both heads (PSUM accumulation groups
must never share a bank). The softmax denominator rides the AV matmuls as an
extra ones-column shared by both heads: V is stored [v_even|ones|pad|v_odd]
per (k-tile, head-pair) so par0 (cols 0:65) puts Z_e at psum partition 64
and par1 (cols 64:192) puts Z_o at partition 0 (matmul PSUM outputs must
start at partition 0 or 64). AV runs par-MAJOR into two single-bank psum
tiles so Z_e's reciprocal + broadcast + y_e staging overlap the par1 stream.
1/Z rows are broadcast with 0-stride-source SBUF->SBUF DMAs (no engine cost,
latency hidden by the deferred norm multiply) and the final scale is one
bf16 DVE multiply; the chain-critical last-chunk norms use stream_shuffle /
Pool-queue DMAs instead so nothing head-blocks SP.SEQ before the tail
output DMAs. The V bias rides the softmax identity (y+b*Z)/Z = y/Z + b,
i.e. it is folded into bproj on the host, so the V psum drain is a pure
strided copy.

Engine balance: PE does matmuls + RoPE permutes + a pstate-ramp warmup
chain on a DVE-memset tile (a raw InstMemset pinned to DVE - the any-engine
memset would land on Pool behind the framework preamble); Act does exp,
early-chunk qkv bias-adds (per-partition bias AP) and y staging copies; DVE
does RoPE multiplies, causal masks (bf16 2x), reciprocals, V drains and
normalize multiplies; the gpsimd SWDGE queue carries the aux input DMAs
(rope tables, bias, mask, wproj) so their descriptor generation runs
parallel to the critical wqk/x/wv stream on the shared HWDGE.

Chunk 0's qkv runs o-major so matmuls chase the startup DMA stream (paired
wqk/x slices of the contraction dim; HWDGE descriptor generation at
~625ns/DMA is the startup pacer). Later chunks prefetch x and interleave
next-chunk qkv between a head pair's AV stream and its normalize; ropes are
deferred one fb behind their bias drains so the perm matmul never stalls on
bias latency. The whole schedule minimizes false WAR stalls from whole-tile
dependency tracking: proj stops are grouped before their copies, 2-bank psum
tiles' second halves are spaced behind their first half's copy, and pool
sizes are large enough that slot reuse never reaches back into live chains.

The last chunk is fused: its hp0 S/exp stream is hoisted into chunk 2's
window, then AV(3,0)-par0 interleaves with S(3,1) emission (so the tail's
Act-bound exp stream starts ~6us early) and AV(3,0)-par1 with AV(3,1)-par0.
The projection is split into pt2 phases across all 8 PSUM banks so output
copies/DMAs drain while hp1's normalize completes, and the tail output DMAs
are split across the sync (HWDGE) and gpsimd (SWDGE) queues so the last
DMA's descriptor generation starts the moment its copy lands.

Timeline (TimelineSim): 161.5us baseline -> 138.7us; PE busy ~120us (86%).
"""
import sys

if "/opt/trn_rl_repo" not in sys.path:
    sys.path.insert(0, "/opt/trn_rl_repo")

import numpy as np
import ml_dtypes

B, T, D = 2, 2048, 1024
H, DH = 16, 64
ROPE_BASE = 10000.0
NCORES = 8
TP = 4                # TP group size (cores per batch)
HL = H // TP          # heads per core = 4
CHUNK = 512           # t/q chunk
NCH = T // CHUNK      # 4
KT = 128              # k tile
NKT = T // KT         # 16
DIN = HL * DH         # 256 local head dims
SCALE = 1.0 / float(np.sqrt(DH))
NWARM = 36            # PE warmup matmuls (pstate ramp)

_compiled = None
_last_results = None


def _build(debug=False):
    import concourse.bass as bass
    import concourse.mybir as mybir
    import concourse.tile as tile
    from concourse import bacc

    F32 = mybir.dt.float32
    BF16 = mybir.dt.bfloat16
    ADD = mybir.AluOpType.add
    MULT = mybir.AluOpType.mult
    EXP = mybir.ActivationFunctionType.Exp
    IDENT = mybir.ActivationFunctionType.Identity

    nc = bacc.Bacc("TRN2", target_bir_lowering=False, num_devices=NCORES)

    xT = nc.dram_tensor("xT", [D, T], BF16, kind="ExternalInput")
    wqk = nc.dram_tensor("wqk", [D, 2 * DIN], BF16, kind="ExternalInput")
    wv = nc.dram_tensor("wv", [D, DIN], BF16, kind="ExternalInput")
    wproj = nc.dram_tensor("wproj", [DIN, D], BF16, kind="ExternalInput")
    bqk = nc.dram_tensor("bqk", [128, 4], F32, kind="ExternalInput")
    cos2 = nc.dram_tensor("cos2", [128, T], BF16, kind="ExternalInput")
    sin2 = nc.dram_tensor("sin2", [128, T], F32, kind="ExternalInput")
    perm = nc.dram_tensor("perm", [128, 128], BF16, kind="ExternalInput")
    trimask = nc.dram_tensor("trimask", [128, 2, 128], BF16, kind="ExternalInput")
    yT = nc.dram_tensor("yT", [D, T], BF16, kind="ExternalOutput")
    if debug:
        dbg_qk = nc.dram_tensor("dbg_qk", [128, 4, T], BF16, kind="ExternalOutput")
        dbg_v = nc.dram_tensor("dbg_v", [128, NKT, 2, 192], BF16, kind="ExternalOutput")
        dbg_y = nc.dram_tensor("dbg_y", [128, 2, T], F32, kind="ExternalOutput")
        dbg_rr = nc.dram_tensor("dbg_rr", [96, CHUNK], F32, kind="ExternalOutput")
        dbg_bc = nc.dram_tensor("dbg_bc", [128, CHUNK], F32, kind="ExternalOutput")
        dbg_yc = nc.dram_tensor("dbg_yc", [128, CHUNK], F32, kind="ExternalOutput")

    xT3 = xT[:].rearrange("(o p) t -> p o t", p=128)
    wqk3 = wqk[:].rearrange("(o p) f -> p o f", p=128)
    wv3 = wv[:].rearrange("(o p) f -> p o f", p=128)

    with tile.TileContext(nc) as tc:
        with tc.tile_pool(name="const", bufs=1) as constp, \
             tc.tile_pool(name="big", bufs=1) as bigp, \
             tc.tile_pool(name="xin", bufs=3) as xinp, \
             tc.tile_pool(name="ptile", bufs=36) as ptp, \
             tc.tile_pool(name="tmp", bufs=5) as tmpp, \
             tc.tile_pool(name="rsm", bufs=6) as rsmp, \
             tc.tile_pool(name="outs", bufs=8) as outsp, \
             tc.tile_pool(name="psmm", bufs=2, space="PSUM") as psmm, \
             tc.tile_pool(name="pss", bufs=2, space="PSUM") as pss, \
             tc.tile_pool(name="psav", bufs=2, space="PSUM") as psav:

            # ---- persistent SBUF tensors ----
            warm_sb = constp.tile([128, 128], BF16)
            wqk_sb = constp.tile([128, 8, 2 * DIN], BF16)     # [p, din_o, f]
            wv_sb = constp.tile([128, 8, DIN], BF16)
            wproj_sb = constp.tile([128, 2, D], BF16)         # [p, din_tile, dout]
            bqk_sb = constp.tile([128, 4], F32)
            cos_sb = constp.tile([128, T], BF16)
            sin_sb = constp.tile([128, T], F32)
            perm_sb = constp.tile([128, 128], BF16)
            tri_sb = constp.tile([128, 2, 128], BF16)

            qk_sb = bigp.tile([128, 4, T], BF16)              # fb: q01,q23,k01,k23
            # per (kt, hp): [v_even(0:64), ones(64), pad(65:128), v_odd
            # (128:192)]. The shared ones column gives Z_e at psum p64 (par0
            # = cols 0:65 -> p0:65) and Z_o at p0 (par1 = cols 64:192 ->
            # p0:128; the uninitialized pad cols feed junk partitions 1:64
            # that are never read). PSUM matmul outputs must start at
            # partition 0 or 64 (a base-32 start is limited to 32 rows).
            v_sb = bigp.tile([128, NKT, 2, 192], BF16)
            y_sb = bigp.tile([128, 2, T], BF16)               # y^T (din on partitions)

            # PE pstate warmup: dep-free matmul chain starts ~150ns in. The
            # init memset is emitted as a raw InstMemset pinned to DVE --
            # nc.*.memset lowers to an any-engine op that the scheduler puts
            # on Pool BEHIND the framework's preamble memsets (~1us late).
            nc.vector.add_instruction(mybir.InstMemset(
                name=nc.get_next_instruction_name(),
                mode="Const", constant=15872,  # bf16 0.125
                ins=[], outs=[nc.vector.lower_ap(warm_sb[:])]))
            pwm = psmm.tile([128, 128], F32, tag="mm", name="pwm")
            for _ in range(NWARM):
                nc.tensor.matmul(pwm[:], warm_sb[:], warm_sb[:], start=True,
                                 stop=True)

            # startup is a 2-stream DMA race: the small rope/bias/mask tables
            # go first on the gpsimd SWDGE queue; the big wqk/x/wv stream
            # feeds chunk 0's o-major qkv from the HWDGE queue in
            # consumption order
            nc.gpsimd.dma_start(perm_sb[:], perm[:])
            nc.gpsimd.dma_start(bqk_sb[:], bqk[:])
            nc.gpsimd.dma_start(cos_sb[:, 0:CHUNK], cos2[:, 0:CHUNK])
            nc.gpsimd.dma_start(sin_sb[:, 0:CHUNK], sin2[:, 0:CHUNK])
            nc.gpsimd.dma_start(tri_sb[:], trimask[:])
            nc.vector.memset(v_sb[:, :, :, 64:65], 1.0)  # softmax-Z ones col
            x_tiles = {}
            x_tiles[0] = xinp.tile([128, 8, CHUNK], BF16, tag="xchunk", name="x_c0")
            for o2 in range(4):
                o = bass.ds(2 * o2, 2)
                nc.sync.dma_start(wqk_sb[:, o], wqk3[:, o])
                nc.sync.dma_start(x_tiles[0][:, o], xT3[:, o, 0:CHUNK])
            for h in range(2):
                nc.sync.dma_start(wv_sb[:, bass.ds(4 * h, 4)],
                                  wv3[:, bass.ds(4 * h, 4)])

            def emit_tail_dmas():
                # late-use aux loads, still on the SWDGE queue
                nc.gpsimd.dma_start(cos_sb[:, CHUNK:], cos2[:, CHUNK:])
                nc.gpsimd.dma_start(sin_sb[:, CHUNK:], sin2[:, CHUNK:])
                nc.gpsimd.dma_start(wproj_sb[:],
                                    wproj[:].rearrange("(o p) f -> p o f", p=128))

            def emit_rope(c, fb, qkpre):
                # roped = qkpre*cos + perm(qkpre)*sin, written to qk_sb
                cc = bass.ds(c * CHUNK, CHUNK)
                pp = psmm.tile([128, CHUNK], F32, tag="mm", name="pp")
                nc.tensor.matmul(pp[:], perm_sb[:], qkpre[:], start=True, stop=True)
                nc.vector.tensor_tensor(qk_sb[:, fb, cc], qkpre[:], cos_sb[:, cc], MULT)
                swapped = tmpp.tile([128, CHUNK], BF16, tag="rope")
                nc.vector.tensor_tensor(swapped[:], pp[:], sin_sb[:, cc], MULT)
                nc.vector.tensor_tensor(qk_sb[:, fb, cc], qk_sb[:, fb, cc], swapped[:], ADD)

            # ropes are deferred one step behind their bias drains so the
            # perm matmul never sits on the PE right after its own pq stop
            # (the bias latency would stall the PE); flushed at the start of
            # the next PE-heavy block
            pending_ropes = []

            def flush_ropes():
                while pending_ropes:
                    emit_rope(*pending_ropes.pop(0))

            def emit_bias(qkpre, pq, fb, act):
                # psum->sbuf drain with the qk bias riding it. Act while its
                # exp stream is still light (early chunks), DVE afterwards.
                if act:
                    nc.scalar.activation(qkpre[:], pq, IDENT,
                                         bias=bqk_sb[:, fb:fb + 1], scale=1.0)
                else:
                    nc.vector.tensor_scalar_add(qkpre[:], pq,
                                                bqk_sb[:, fb:fb + 1])

            def emit_vwrite(kt, hp, pv):
                # pure strided copy (V bias is folded into bproj on the host):
                # pv cols [hp*128, hp*128+128) -> v_sb cols {0:64, 128:192}
                vdst = v_sb[:, kt, hp].rearrange("p (a b) -> p a b", b=64)[:, 0::2]
                vsrc = pv[:, hp * 128:(hp + 1) * 128].rearrange("p (a b) -> p a b", b=64)
                nc.vector.tensor_copy(vdst, vsrc)

            def emit_qkv0():
                # chunk 0: o-major so matmuls chase the startup DMA stream
                x_sb = x_tiles.pop(0)
                # borrow the attention-stage pss slots (idle during startup):
                # two 2-bank slots hold the four q/k blocks, one per bank
                # (PSUM accumulation groups must not share a bank)
                psq = [pss.tile([128, 2, CHUNK], F32, tag="s", name=f"psq{i}")
                       for i in range(2)]
                pqs = [psq[fb // 2][:, fb % 2, :] for fb in range(4)]
                for o in range(8):
                    for fb in range(4):
                        nc.tensor.matmul(
                            pqs[fb], wqk_sb[:, o, fb * 128:(fb + 1) * 128],
                            x_sb[:, o], start=(o == 0), stop=(o == 7),
                            skip_group_check=True)
                def finish(fbs, tbs):
                    for fb in fbs:
                        qkpre = tmpp.tile([128, CHUNK], BF16, tag="qkpre")
                        emit_bias(qkpre, pqs[fb], fb, act=True)
                        emit_rope(0, fb, qkpre)
                    for tb in tbs:
                        pvfull = psmm.tile([128, CHUNK], F32, tag="mm",
                                           name="pvfull")
                        pv = pvfull[:, :DIN]
                        for o in range(8):
                            nc.tensor.matmul(
                                pv[:], x_sb[:, o, tb * 128:(tb + 1) * 128],
                                wv_sb[:, o], start=(o == 0), stop=(o == 7))
                        for hp in range(2):
                            emit_vwrite(tb, hp, pv[:])
                return finish

            def emit_qkv_qk(c, fbs=range(4)):
                # q^T,k^T for chunk c: [f, t], bias-add + deferred rope
                x_sb = x_tiles[c]
                for fb in fbs:
                    pq = psmm.tile([128, CHUNK], F32, tag="mm", name="pq")
                    for o in range(8):
                        nc.tensor.matmul(
                            pq[:], wqk_sb[:, o, fb * 128:(fb + 1) * 128], x_sb[:, o],
                            start=(o == 0), stop=(o == 7))
                    qkpre = tmpp.tile([128, CHUNK], BF16, tag="qkpre")
                    emit_bias(qkpre, pq[:], fb, act=(c <= 1))
                    pending_ropes.append((c, fb, qkpre))
                    if len(pending_ropes) > 1:
                        emit_rope(*pending_ropes.pop(0))

            def emit_qkv_v(c):
                x_sb = x_tiles.pop(c)
                for tb in range(4):
                    if tb == 1:
                        flush_ropes()
                    kt = c * 4 + tb
                    pvfull = psmm.tile([128, CHUNK], F32, tag="mm", name="pvfull")
                    pv = pvfull[:, :DIN]
                    for o in range(8):
                        nc.tensor.matmul(
                            pv[:], x_sb[:, o, tb * 128:(tb + 1) * 128], wv_sb[:, o],
                            start=(o == 0), stop=(o == 7))
                    for hp in range(2):
                        emit_vwrite(kt, hp, pv[:])

            def emit_xload(c, slices=1):
                if c < NCH:
                    x_tiles[c] = xinp.tile([128, 8, CHUNK], BF16, tag="xchunk",
                                           name=f"x_c{c}")
                    cc = bass.ds(c * CHUNK, CHUNK)
                    w = 8 // slices
                    for i in range(slices):
                        nc.sync.dma_start(x_tiles[c][:, i * w:(i + 1) * w],
                                          xT3[:, i * w:(i + 1) * w, cc])

            def emit_attn_S(c, hp):
                # S + exp + mask for q-chunk c, head pair hp; even head uses
                # PE rows 0-63 / psum par 0, odd head rows 64-127 / par 1.
                nkt_c = 4 * c + 4
                p_tiles = []
                for kt in range(nkt_c):
                    if kt == 1:
                        flush_ropes()
                    i = kt - 4 * c  # >=0 on diagonal tiles
                    col0 = 128 * i if i >= 0 else 0
                    ps = pss.tile([128, 2, CHUNK], F32, tag="s", name="ps")
                    for par in range(2):
                        base = 64 * par
                        nc.tensor.matmul(
                            ps[:, par, col0:],
                            qk_sb[base:base + 64, 2 + hp, kt * 128:(kt + 1) * 128],
                            qk_sb[base:base + 64, hp, bass.ds(c * CHUNK + col0,
                                                              CHUNK - col0)],
                            start=True, stop=True, skip_group_check=True)
                    pt = ptp.tile([128, 2, CHUNK], BF16, tag="p", name="pt")
                    nc.scalar.activation(pt[:, :, col0:], ps[:, :, col0:], EXP,
                                         bias=0.0, scale=SCALE)
                    if i >= 0:
                        # zero k>q entries of the diagonal block (0/1 mask);
                        # cheap on DVE in bf16 2x mode (~200ns vs ~600 Pool)
                        nc.vector.tensor_tensor(
                            pt[:, :, col0:col0 + 128], pt[:, :, col0:col0 + 128],
                            tri_sb[:], MULT)
                    p_tiles.append(pt)
                return p_tiles

            def emit_attn_AV(c, hp, p_tiles):
                # par-MAJOR: all of par0's AV accumulation first, so Z_e's
                # reciprocal + broadcast + y_e staging overlap the par1 matmul
                # stream; only par1's short chain is exposed at the end.
                nkt_c = 4 * c + 4
                last = c == NCH - 1
                pav0 = psav.tile([128, CHUNK], F32, tag="av", name="pav0")
                pav1 = psav.tile([128, CHUNK], F32, tag="av", name="pav1")
                rr2 = rsmp.tile([96, CHUNK], BF16, tag="r", name="rr2")
                yc = rsmp.tile([128, CHUNK], BF16, tag="yc", name="yc")
                bc_sb = rsmp.tile([128, CHUNK], BF16, tag="bc", name="bc_sb")

                def stream(par):
                    for kt in range(nkt_c):
                        i = kt - 4 * c
                        col0 = 128 * i if i >= 0 else 0
                        pt = p_tiles[kt]
                        if par == 0:
                            nc.tensor.matmul(
                                pav0[:65, col0:], v_sb[:, kt, hp, 0:65],
                                pt[:, 0, col0:], start=(kt == 0),
                                stop=(kt == nkt_c - 1), skip_group_check=True)
                        else:
                            nc.tensor.matmul(
                                pav1[:, col0:], v_sb[:, kt, hp, 64:192],
                                pt[:, 1, col0:], start=(kt == 0),
                                stop=(kt == nkt_c - 1), skip_group_check=True)

                stream(0)
                with nc.allow_low_precision(reason="1/Z in bf16, ~0.4% rel"):
                    nc.vector.reciprocal(rr2[64:65, :], pav0[64:65, :])
                # 1/Z broadcast = 0-stride-source SBUF->SBUF DMA: no engine
                # cost, ~2.4us latency hidden under the par1 stream / the
                # deferred norm multiply
                nc.sync.dma_start(
                    bc_sb[0:64, :],
                    rr2[64:65, :].rearrange("p (o t) -> p o t", o=1)
                    .to_broadcast((1, 64, CHUNK)))
                nc.scalar.copy(yc[0:64, :], pav0[0:64, :])
                stream(1)
                with nc.allow_low_precision(reason="1/Z in bf16, ~0.4% rel"):
                    nc.vector.reciprocal(rr2[0:1, :], pav1[0:1, :])
                if last and hp == 1:
                    # chain-critical final normalize: stream_shuffle is lower
                    # latency than a DMA round trip
                    nc.vector.stream_shuffle(bc_sb[64:96, :], rr2[0:32, :], [0] * 32)
                    nc.vector.stream_shuffle(bc_sb[96:128, :], rr2[0:32, :], [0] * 32)
                else:
                    nc.sync.dma_start(
                        bc_sb[64:128, :],
                        rr2[0:1, :].rearrange("p (o t) -> p o t", o=1)
                        .to_broadcast((1, 64, CHUNK)))
                if last:
                    nc.scalar.copy(yc[64:128, :], pav1[64:128, :])
                else:
                    nc.vector.tensor_copy(yc[64:128, :], pav1[64:128, :])
                return yc, bc_sb

            def emit_attn_hp(c, hp):
                return emit_attn_AV(c, hp, emit_attn_S(c, hp))

            def emit_norm(c, hp, yc, bc_sb, dump=False):
                # final softmax scale: one full-width bf16 multiply on DVE
                # (2x mode); emitted late so its bc wait never blocks earlier
                # DVE work on the in-order queue
                cc = bass.ds(c * CHUNK, CHUNK)
                nc.vector.tensor_tensor(y_sb[:, hp, cc], yc[:], bc_sb[:], MULT)
                if dump:
                    nc.sync.dma_start(dbg_bc[:], bc_sb[:].bitcast(F32)[:, :CHUNK // 2])
                    nc.sync.dma_start(dbg_yc[:], yc[:].bitcast(F32)[:, :CHUNK // 2])

            def emit_proj_last():
                # attention PSUM is free: all 8 output blocks get their own
                # bank; pt2=0 matmuls (needing only hp0's y) fill the PE while
                # hp1's softmax-normalize drains, pt2=1 + copies follow
                cc = bass.ds((NCH - 1) * CHUNK, CHUNK)
                prm0 = psmm.tile([128, CHUNK], F32, tag="mm", name="prm")
                prm1 = psmm.tile([128, CHUNK], F32, tag="mm", name="prm")
                ps20 = pss.tile([128, 2, CHUNK], F32, tag="s", name="prs0")
                ps21 = pss.tile([128, 2, CHUNK], F32, tag="s", name="prs1")
                prva = psav.tile([128, CHUNK], F32, tag="av", name="prva")
                prvb = psav.tile([128, CHUNK], F32, tag="av", name="prvb")
                # bank -> output block db is fixed; iteration orders differ:
                # phase_a defers the psav banks (blocked on pav(3,1) readers),
                # phase_b spaces a 2-bank tile's second stop 3+ matmuls after
                # its first half's copy (whole-tile WAR tracking would
                # otherwise stall the PE)
                banks = {0: prm0[:], 1: prm1[:], 2: ps20[:, 0, :],
                         3: ps20[:, 1, :], 4: ps21[:, 0, :], 5: ps21[:, 1, :],
                         6: prva[:], 7: prvb[:]}
                a_order = [0, 1, 2, 3, 4, 5, 6, 7]
                b_order = [0, 1, 2, 4, 6, 3, 5, 7]

                def phase_a():
                    for db in a_order:
                        nc.tensor.matmul(
                            banks[db], wproj_sb[:, 0, db * 128:(db + 1) * 128],
                            y_sb[:, 0, cc], start=True, stop=False,
                            skip_group_check=True)

                def phase_b():
                    for pos, db in enumerate(b_order):
                        nc.tensor.matmul(
                            banks[db], wproj_sb[:, 1, db * 128:(db + 1) * 128],
                            y_sb[:, 1, cc], start=False, stop=True,
                            skip_group_check=True)
                        o_sb = outsp.tile([128, CHUNK], BF16, tag="o")
                        # alternate copy engines by EMISSION position so the
                        # final copies never serialize on one engine; the two
                        # LAST banks' copies are split Act/DVE half-width so
                        # both engines drain them in parallel. Early banks go
                        # out on the slow SWDGE (Pool) queue, late banks on
                        # HWDGE (625 vs 1038ns gen): the LAST DMA's gen must
                        # not queue behind three 1038ns SWDGE gens.
                        if pos >= 6:
                            nc.scalar.copy(o_sb[:, 0:CHUNK // 2],
                                           banks[db][:, 0:CHUNK // 2])
                            nc.vector.tensor_copy(o_sb[:, CHUNK // 2:],
                                                  banks[db][:, CHUNK // 2:])
                        elif pos % 2 == 0:
                            nc.scalar.copy(o_sb[:], banks[db])
                        else:
                            nc.vector.tensor_copy(o_sb[:], banks[db])
                        # pos 1-6 share the HWDGE gen chain; pos 0 and the
                        # LAST bank get the otherwise-idle SWDGE queue so the
                        # final DMA's gen starts the moment its copy lands
                        eng = nc.gpsimd if pos in (0, 7) else nc.sync
                        eng.dma_start(yT[db * 128:(db + 1) * 128, cc], o_sb[:])
                return phase_a, phase_b

            def emit_proj(c):
                # pr slots alternate between the (idle-here) psav 2-bank slot
                # and psmm singles; all 4 stops of a quad are issued before
                # its copies so whole-tile WAR tracking on the 2-bank psav
                # tile never stalls the PE
                cc = bass.ds(c * CHUNK, CHUNK)
                for quad in range(2):
                    prs = [psav.tile([128, CHUNK], F32, tag="av", name="prv")[:]
                           for _ in range(2)]
                    for j in range(2):
                        prm = psmm.tile([128, CHUNK], F32, tag="mm", name="prm")
                        prs.append(prm[:])
                    for j, pr in enumerate(prs):
                        db = quad * 4 + j
                        for pt2 in range(2):
                            nc.tensor.matmul(
                                pr, wproj_sb[:, pt2, db * 128:(db + 1) * 128],
                                y_sb[:, pt2, cc], start=(pt2 == 0), stop=(pt2 == 1),
                                skip_group_check=True)
                    for j, pr in enumerate(prs):
                        db = quad * 4 + j
                        o_sb = outsp.tile([128, CHUNK], BF16, tag="o")
                        if db % 2 == 0 and c != 2:
                            nc.scalar.copy(o_sb[:], pr)
                        else:
                            nc.vector.tensor_copy(o_sb[:], pr)
                        nc.sync.dma_start(yT[db * 128:(db + 1) * 128, cc], o_sb[:])

            # software pipeline: next chunk's qkv matmuls sit between a head
            # pair's AV stream and its softmax-normalize so the PE never waits
            # on the reciprocal; proj work always trails norms.
            finish0 = emit_qkv0()
            emit_xload(1, slices=2)
            emit_tail_dmas()
            finish0([0, 2], [0, 1, 2, 3])  # rope q01/k01 + all chunk-0 V
            emit_qkv_qk(1, [0, 1])      # PE filler while DVE ropes chunk 0
            for c in range(NCH):
                st0 = emit_attn_hp(c, 0) if c < NCH - 1 else None
                if c == 0:
                    finish0([1, 3], [])  # rope q23/k23 under hp0's S
                if c + 1 < NCH:
                    emit_xload(c + 2)
                    emit_qkv_qk(c + 1, [2, 3] if c == 0 else [0, 2])
                if c + 1 < NCH:
                    emit_norm(c, 0, *st0)
                    st1 = emit_attn_hp(c, 1)
                    if c > 0:
                        emit_qkv_qk(c + 1, [1, 3])
                    emit_qkv_v(c + 1)
                    emit_norm(c, 1, *st1)
                    if c + 1 == NCH - 1:
                        # feed the Act-bound last chunk early: its hp0 S/exp
                        # stream interleaves with this chunk's proj
                        p3_0 = emit_attn_S(c + 1, 0)
                    emit_proj(c)
                else:
                    # ---- fused last chunk ----
                    # The tail is Act(exp)-bound: S(3,1)'s 16 exps pace
                    # everything. Interleave AV(3,0)-par0 with S(3,1) so the
                    # exp stream starts ~6us earlier, then AV(3,0)-par1 with
                    # AV(3,1)-par0. Masks go to Pool and the hp0 chains to
                    # DVE so nothing queues behind the exp stream on Act.
                    nkt_c = 4 * c + 4
                    pav0_0 = psav.tile([128, CHUNK], F32, tag="av", name="pav0")
                    pav1_0 = psav.tile([128, CHUNK], F32, tag="av", name="pav1")
                    rr_0 = rsmp.tile([96, CHUNK], BF16, tag="r", name="rr2")
                    yc_0 = rsmp.tile([128, CHUNK], BF16, tag="yc", name="yc")
                    bc_0 = rsmp.tile([128, CHUNK], BF16, tag="bc", name="bc_sb")

                    def av_mm(hp, par, pav, p_tiles, kt):
                        i = kt - 4 * c
                        col0 = 128 * i if i >= 0 else 0
                        pt = p_tiles[kt]
                        if par == 0:
                            nc.tensor.matmul(
                                pav[:65, col0:], v_sb[:, kt, hp, 0:65],
                                pt[:, 0, col0:], start=(kt == 0),
                                stop=(kt == nkt_c - 1), skip_group_check=True)
                        else:
                            nc.tensor.matmul(
                                pav[:, col0:], v_sb[:, kt, hp, 64:192],
                                pt[:, 1, col0:], start=(kt == 0),
                                stop=(kt == nkt_c - 1), skip_group_check=True)

                    p3_1 = []
                    for kt in range(nkt_c):
                        av_mm(0, 0, pav0_0, p3_0, kt)
                        i = kt - 4 * c
                        col0 = 128 * i if i >= 0 else 0
                        ps = pss.tile([128, 2, CHUNK], F32, tag="s", name="ps")
                        for par in range(2):
                            base = 64 * par
                            nc.tensor.matmul(
                                ps[:, par, col0:],
                                qk_sb[base:base + 64, 3, kt * 128:(kt + 1) * 128],
                                qk_sb[base:base + 64, 1,
                                      bass.ds(c * CHUNK + col0, CHUNK - col0)],
                                start=True, stop=True, skip_group_check=True)
                        pt = ptp.tile([128, 2, CHUNK], BF16, tag="p", name="pt")
                        nc.scalar.activation(pt[:, :, col0:], ps[:, :, col0:],
                                             EXP, bias=0.0, scale=SCALE)
                        if i >= 0:
                            nc.gpsimd.tensor_tensor(
                                pt[:, :, col0:col0 + 128],
                                pt[:, :, col0:col0 + 128], tri_sb[:], MULT)
                        p3_1.append(pt)
                    with nc.allow_low_precision(reason="1/Z in bf16"):
                        nc.vector.reciprocal(rr_0[64:65, :], pav0_0[64:65, :])
                    # last-chunk bc halves ride the Pool SWDGE queue: a
                    # waiting sync-queue DMA would head-block SP.SEQ and
                    # delay the tail output DMAs queued behind it
                    nc.gpsimd.dma_start(
                        bc_0[0:64, :],
                        rr_0[64:65, :].rearrange("p (o t) -> p o t", o=1)
                        .to_broadcast((1, 64, CHUNK)))
                    nc.vector.tensor_copy(yc_0[0:64, :], pav0_0[0:64, :])

                    pav0_1 = psav.tile([128, CHUNK], F32, tag="av", name="pav0")
                    for kt in range(nkt_c):
                        av_mm(0, 1, pav1_0, p3_0, kt)
                        av_mm(1, 0, pav0_1, p3_1, kt)
                    with nc.allow_low_precision(reason="1/Z in bf16"):
                        nc.vector.reciprocal(rr_0[0:1, :], pav1_0[0:1, :])
                    nc.gpsimd.dma_start(
                        bc_0[64:128, :],
                        rr_0[0:1, :].rearrange("p (o t) -> p o t", o=1)
                        .to_broadcast((1, 64, CHUNK)))
                    nc.vector.tensor_copy(yc_0[64:128, :], pav1_0[64:128, :])
                    rr_1 = rsmp.tile([96, CHUNK], BF16, tag="r", name="rr2")
                    yc_1 = rsmp.tile([128, CHUNK], BF16, tag="yc", name="yc")
                    bc_1 = rsmp.tile([128, CHUNK], BF16, tag="bc", name="bc_sb")
                    with nc.allow_low_precision(reason="1/Z in bf16"):
                        nc.vector.reciprocal(rr_1[64:65, :], pav0_1[64:65, :])
                    nc.gpsimd.dma_start(
                        bc_1[0:64, :],
                        rr_1[64:65, :].rearrange("p (o t) -> p o t", o=1)
                        .to_broadcast((1, 64, CHUNK)))
                    nc.scalar.copy(yc_1[0:64, :], pav0_1[0:64, :])
                    emit_norm(c, 0, yc_0, bc_0, dump=debug)

                    pav1_1 = psav.tile([128, CHUNK], F32, tag="av", name="pav1")
                    for kt in range(nkt_c):
                        av_mm(1, 1, pav1_1, p3_1, kt)
                    with nc.allow_low_precision(reason="1/Z in bf16"):
                        nc.vector.reciprocal(rr_1[0:1, :], pav1_1[0:1, :])
                    nc.vector.stream_shuffle(bc_1[64:96, :], rr_1[0:32, :], [0] * 32)
                    nc.vector.stream_shuffle(bc_1[96:128, :], rr_1[0:32, :], [0] * 32)
                    nc.scalar.copy(yc_1[64:128, :], pav1_1[64:128, :])
                    pa, pb = emit_proj_last()
                    pa()
                    emit_norm(c, 1, yc_1, bc_1)
                    pb()

            if debug:
                nc.sync.dma_start(dbg_qk[:], qk_sb[:])
                nc.sync.dma_start(dbg_v[:], v_sb[:])
                nc.sync.dma_start(dbg_y[:], y_sb[:].bitcast(F32))

    nc.finalize()
    return nc


def _host_inputs(x, Wqkv, bqkv, Wproj):
    """Per-core input maps. Core c: batch c//TP, heads [4*(c%TP), 4*(c%TP)+4)."""
    BF = ml_dtypes.bfloat16
    # RoPE tables in ^T layout, rows = head-local dim d (pattern repeats each 64)
    d = np.arange(64)
    inv_freq = 1.0 / (ROPE_BASE ** (np.arange(0, DH, 2, dtype=np.float64) / DH))  # [32]
    ang = np.arange(T, dtype=np.float64)[None, :] * inv_freq[d // 2][:, None]     # [64, T]
    cos64 = np.cos(ang)
    sin64 = np.sin(ang) * np.where(d % 2 == 0, -1.0, 1.0)[:, None]
    cos2 = np.tile(cos64, (2, 1)).astype(BF)
    sin2 = np.tile(sin64, (2, 1)).astype(np.float32)

    perm = np.zeros((128, 128), np.float32)
    perm[np.arange(128) ^ 1, np.arange(128)] = 1.0

    ki, qi = np.meshgrid(np.arange(128), np.arange(128), indexing="ij")
    tri = np.where(ki <= qi, 1.0, 0.0).astype(BF)
    tri3 = np.ascontiguousarray(np.broadcast_to(tri[:, None, :], (128, 2, 128)))

    Wq, Wk = Wqkv[:, :D], Wqkv[:, D:2 * D]
    Wv = Wqkv[:, 2 * D:]
    bq, bk = bqkv[:D], bqkv[D:2 * D]

    maps = []
    for core in range(NCORES):
        b, r = core // TP, core % TP
        sl = slice(r * DIN, (r + 1) * DIN)
        wqk_c = np.concatenate([Wq[:, sl], Wk[:, sl]], axis=1)
        bqk_c = np.concatenate([bq[sl], bk[sl]]).astype(np.float32)
        maps.append({
            "xT": np.ascontiguousarray(x[b].T).astype(BF),
            "wqk": wqk_c.astype(BF),
            "wv": np.ascontiguousarray(Wv[:, sl]).astype(BF),
            "wproj": np.ascontiguousarray(Wproj[sl, :]).astype(BF),
            "bqk": np.ascontiguousarray(bqk_c.reshape(4, 128).T),
            "cos2": cos2,
            "sin2": sin2,
            "perm": perm.astype(BF),
            "trimask": tri3,
        })
    return maps


def kernel(x, Wqkv, bqkv, Wproj, bproj):
    global _compiled, _last_results
    from concourse.bass_utils import run_bass_kernel_spmd

    if _compiled is None:
        _compiled = _build()
    nc = _compiled

    x = np.asarray(x, np.float32)
    Wqkv = np.asarray(Wqkv, np.float32)
    bqkv = np.asarray(bqkv, np.float32)
    Wproj = np.asarray(Wproj, np.float32)
    maps = _host_inputs(x, Wqkv, bqkv, Wproj)
    res = run_bass_kernel_spmd(nc, maps, core_ids=list(range(NCORES)))
    _last_results = res
    # V bias rides the softmax identity (y+b*Z)/Z = y/Z+b: fold bv@Wproj into
    # the output bias (exact for any bqkv)
    bv = np.asarray(bqkv, np.float64)[2 * D:]
    bproj_eff = np.asarray(bproj, np.float64) + bv @ np.asarray(Wproj, np.float64)
    out = np.empty((B, T, D), np.float32)
    for b in range(B):
        acc = np.zeros((D, T), np.float64)
        for r in range(TP):
            acc += np.asarray(res.results[b * TP + r]["yT"], dtype=np.float64)
        out[b] = acc.T + bproj_eff[None, :]
    return out
